# revision 12
# baseline (speedup 1.0000x reference)
"""Trainium2 Bass kernel for a GCN-based DQN forward pass (8 NeuronCores).

v2 strategy (dst-sharded nodes+edges, one-hot scatter matmuls):
 - host folds W_e1/b_e1 into a single f32 stream s0 = a*src + b*dst + d
 - unified degree pass (slot layout) -> deg -> dis = 1/sqrt(deg+1)
 - local table shard: dis * (x @ W_gcn) in bf16, kept in SBUF for the
   self-loop term; written to DRAM and AllGathered in TWO rank-halves so
   gathers can start after the first collective
 - spine: dma_gather of per-edge source rows on 4 SWDGE queues
   (round-robin) -> one-hot scatter matmuls per 128-edge block, one-hot
   built 3:1 on DVE (is_eq+mult) / ACT (square + relu trick); padding
   edges carry dstloc=-1 so their one-hot column is zero
 - finalize per window: dis_dst scaling + bias + relu, pooling matmuls,
   AllReduce of pooled sums/counts, replicated tiny MLP head
"""
import numpy as np
import ml_dtypes

BF16 = ml_dtypes.bfloat16


def _default_cfg():
    return dict(N=50000, E=1600000, G=64, A=8, NCORES=8, WIN=49, GRP=7)


def _derived(cfg):
    c = dict(cfg)
    c["SH_REAL"] = -(-c["N"] // c["NCORES"])          # real nodes per core (ceil)
    c["SH"] = c["WIN"] * 128                          # padded nodes per core
    assert c["SH"] >= c["SH_REAL"]
    assert c["SH"] % 2 == 0
    c["SHH"] = c["SH"] // 2                           # rank-half size
    c["NTOTH"] = c["NCORES"] * c["SHH"]               # rows per half table
    assert c["NTOTH"] - 1 <= 32767, "half-table must be int16-indexable"
    c["CALLB"] = 16                                   # blocks per gather call
    return c


def _prep(cfg, x, edge_attr, W_e1, b_e1, W_e2, b_e2, W_gcn, b_gcn, W2, b2, W3, b3,
          edge_index, batch):
    """Host-side sharding/layout. Returns (in_maps, meta)."""
    N, E, G, A = cfg["N"], cfg["E"], cfg["G"], cfg["A"]
    NC, WIN, SH_REAL, SH = cfg["NCORES"], cfg["WIN"], cfg["SH_REAL"], cfg["SH"]
    SHH, CALLB = cfg["SHH"], cfg["CALLB"]

    x = np.asarray(x, np.float32)
    edge_attr = np.asarray(edge_attr, np.float32)
    edge_index = np.asarray(edge_index)
    batch = np.asarray(batch)
    src = np.asarray(edge_index[0], np.int64)
    dst = np.asarray(edge_index[1], np.int64)
    attr = edge_attr[:, 0]

    deg = np.bincount(dst, minlength=N)

    # per-core degree-sorted window/slot assignment
    node_of_rank = np.full((NC, SH), -1, np.int64)   # rank -> orig node id (-1 pad)
    rank_of_orig = np.empty(N, np.int64)             # orig -> rank within its core
    R1_cw = np.zeros((NC, WIN), np.int64)
    for c in range(NC):
        lo, hi = c * SH_REAL, min((c + 1) * SH_REAL, N)
        nreal = hi - lo
        d_loc = np.full(SH, -1, np.int64)
        d_loc[:nreal] = deg[lo:hi]
        order = np.argsort(-d_loc, kind="stable")    # rank -> padded-loc
        rank = np.empty(SH, np.int64)
        rank[order] = np.arange(SH)
        node_of_rank[c] = np.where(order < nreal, lo + order, -1)
        rank_of_orig[lo:hi] = rank[:nreal]
        R1_cw[c] = np.maximum(d_loc[order].reshape(WIN, 128), 0).max(axis=1)

    R1_w = R1_cw.max(axis=0)
    R1TOT = max(int(R1_w.sum()), 1)
    woff1 = np.zeros(WIN + 1, np.int64)
    woff1[1:] = np.cumsum(R1_w)

    core_of = np.minimum(np.arange(N) // SH_REAL, NC - 1)

    # per-edge coordinates
    ecore = np.minimum(dst // SH_REAL, NC - 1)
    erank = rank_of_orig[dst]
    ew = erank // 128
    ep = erank % 128
    score = np.minimum(src // SH_REAL, NC - 1)
    srank = rank_of_orig[src]
    ehalf = (srank >= SHH).astype(np.int64)
    erow = score * SHH + srank - ehalf * SHH         # row within half table

    # j1 = rank of edge within its dst-node's list (degree pass)
    eorder = np.argsort(dst, kind="stable")
    starts = np.zeros(N + 1, np.int64)
    starts[1:] = np.cumsum(deg)
    j1 = np.empty(E, np.int64)
    j1[eorder] = np.arange(E) - starts[dst[eorder]]

    # pass-2 segment = (window, half); per-core counts -> uniform block counts
    segid = ew * 2 + ehalf                            # 0..2*WIN-1
    cnt = np.zeros((NC, 2 * WIN), np.int64)
    for c in range(NC):
        m = ecore == c
        cnt[c] = np.bincount(segid[m], minlength=2 * WIN)
    NB_seg = -(-cnt.max(axis=0) // 128)               # blocks per segment (uniform)

    seg_boff = np.zeros(2 * WIN, np.int64)
    calls = []                                        # (half, block_start, nblocks)
    pos = 0
    for w in range(WIN):
        for h in (0, 1):
            seg_boff[w * 2 + h] = pos
            nseg = int(NB_seg[w * 2 + h])
            b = pos
            pos += nseg
            while b < pos:
                nb = min(CALLB, pos - b)
                calls.append((h, int(b), int(nb)))
                b += nb
    NBLK = max(int(pos), 1)

    # per-window block list in call order
    win_blocks = []
    for w in range(WIN):
        blks = []
        for h in (0, 1):
            b0 = int(seg_boff[w * 2 + h])
            blks.extend(range(b0, b0 + int(NB_seg[w * 2 + h])))
        win_blocks.append(blks)

    # j2 = rank of edge within its (core, segment) group
    keys = (ecore * (2 * WIN) + segid)
    eorder2 = np.argsort(keys, kind="stable")
    gcnt = np.bincount(keys, minlength=NC * 2 * WIN)
    gstarts = np.zeros(NC * 2 * WIN + 1, np.int64)
    gstarts[1:] = np.cumsum(gcnt)
    j2 = np.empty(E, np.int64)
    j2[eorder2] = np.arange(E) - gstarts[keys[eorder2]]

    we1 = np.asarray(W_e1, np.float64).reshape(3)
    be1 = float(np.asarray(b_e1, np.float64).reshape(-1)[0])
    we2 = float(np.asarray(W_e2, np.float64).reshape(-1)[0])
    be2 = float(np.asarray(b_e2, np.float64).reshape(-1)[0])
    s0_all = (we1[0] * src + we1[1] * dst + be1).astype(np.float32)

    ecv = np.array([we1[2], we2, be2, 0.0], np.float32)
    ec_bcast = np.ascontiguousarray(np.broadcast_to(ecv, (128, 4)))

    iota128 = np.ascontiguousarray(
        np.broadcast_to(np.arange(128, dtype=np.float32), (128, 128)).astype(BF16))
    bgcn_b = np.ascontiguousarray(
        np.broadcast_to(np.asarray(b_gcn, np.float32), (128, 128)))
    b3_b = np.ascontiguousarray(
        np.broadcast_to(np.asarray(b3, np.float32), (64, A)))
    ident64 = np.eye(64, dtype=BF16)
    wgcn_b16 = np.ascontiguousarray(np.asarray(W_gcn, np.float32)).astype(BF16)
    w2_b16 = np.ascontiguousarray(np.asarray(W2, np.float32)).astype(BF16)
    w3_b16 = np.ascontiguousarray(np.asarray(W3, np.float32)).astype(BF16)
    b2_np = np.ascontiguousarray(np.asarray(b2, np.float32).reshape(128, 1))

    in_maps = []
    for c in range(NC):
        m = ecore == c
        s_s0, s_attr = s0_all[m], attr[m]
        s_ep, s_ew, s_j1, s_j2 = ep[m], ew[m], j1[m], j2[m]
        s_seg, s_row = segid[m], erow[m]

        # degree-pass slot-layout streams [128, R1TOT]
        p1_s0 = np.zeros((128, R1TOT), np.float32)
        p1_attr = np.zeros((128, R1TOT), BF16)
        p1_mask = np.zeros((128, R1TOT), BF16)
        col1 = woff1[s_ew] + s_j1
        p1_s0[s_ep, col1] = s_s0
        p1_attr[s_ep, col1] = s_attr
        p1_mask[s_ep, col1] = 1.0

        # block-layout streams [128, NBLK]
        p2_s0 = np.zeros((128, NBLK), np.float32)
        p2_attr = np.zeros((128, NBLK), BF16)
        p2_dl = np.full((128, NBLK), -1.0, np.float32)  # pads: one-hot column dead
        blk = seg_boff[s_seg] + s_j2 // 128
        pp = s_j2 % 128
        p2_s0[pp, blk] = s_s0
        p2_attr[pp, blk] = s_attr
        p2_dl[pp, blk] = s_ep

        # gather idx stream, wrapped int16 [128, NBLK*8]; pads fetch row 0
        idx_flat = np.zeros(NBLK * 128, np.int64)
        k = blk * 128 + pp
        idx_flat[k] = s_row
        idx16 = np.zeros((128, NBLK * 8), np.int16)
        wrap = idx_flat.reshape(NBLK * 8, 16).T.astype(np.int16)
        for gg in range(8):
            idx16[gg * 16:(gg + 1) * 16, :] = wrap

        # xT in slot order [128, SH] bf16
        nr = node_of_rank[c]
        valid = nr >= 0
        xs = np.zeros((SH, x.shape[1]), np.float32)
        xs[valid] = x[nr[valid]]
        xT = np.ascontiguousarray(xs.T).astype(BF16)

        batch_slot = np.full((128, WIN), 127.0, np.float32)
        nmask = np.zeros((128, WIN), BF16)
        bvals = np.full(SH, 127, np.int64)
        bvals[valid] = batch[nr[valid]]
        batch_slot[:, :] = bvals.reshape(WIN, 128).T
        nmask[:, :] = (valid.reshape(WIN, 128).T).astype(BF16)

        in_maps.append({
            "p1_s0": p1_s0, "p1_attr": p1_attr, "p1_mask": p1_mask,
            "p2_s0": p2_s0, "p2_attr": p2_attr, "p2_dl": p2_dl, "p2_idx": idx16,
            "xT": xT, "batch_slot": batch_slot, "nmask": nmask,
            "iota": iota128, "ecb": ec_bcast, "wgcn": wgcn_b16, "bgcnb": bgcn_b,
            "w2": w2_b16, "b2": b2_np, "w3": w3_b16, "b3b": b3_b, "ident": ident64,
        })

    meta = dict(R1TOT=R1TOT, R1_w=[int(v) for v in R1_w],
                woff1=[int(v) for v in woff1],
                NBLK=NBLK, calls=calls, win_blocks=win_blocks)
    return in_maps, meta


def _build(cfg, meta):
    from concourse import bass, bacc, tile
    import concourse.mybir as mybir

    f32 = mybir.dt.float32
    bf16 = mybir.dt.bfloat16
    i16 = mybir.dt.int16
    Alu = mybir.AluOpType
    Act = mybir.ActivationFunctionType

    NC, WIN, SH, SHH = cfg["NCORES"], cfg["WIN"], cfg["SH"], cfg["SHH"]
    NTOTH, G, A = cfg["NTOTH"], cfg["G"], cfg["A"]
    R1TOT, R1_w, woff1 = meta["R1TOT"], meta["R1_w"], meta["woff1"]
    NBLK, calls, win_blocks = meta["NBLK"], meta["calls"], meta["win_blocks"]

    nc = bacc.Bacc("TRN2", target_bir_lowering=False, debug=False, num_devices=NC,
                   num_swdge_queues=4)

    dram = lambda nm, shp, dt: nc.dram_tensor(nm, shp, dt, kind="ExternalInput")
    p1_s0_d = dram("p1_s0", [128, R1TOT], f32)
    p1_attr_d = dram("p1_attr", [128, R1TOT], bf16)
    p1_mask_d = dram("p1_mask", [128, R1TOT], bf16)
    p2_s0_d = dram("p2_s0", [128, NBLK], f32)
    p2_attr_d = dram("p2_attr", [128, NBLK], bf16)
    p2_dl_d = dram("p2_dl", [128, NBLK], f32)
    p2_idx_d = dram("p2_idx", [128, NBLK * 8], i16)
    xT_d = dram("xT", [128, SH], bf16)
    batch_d = dram("batch_slot", [128, WIN], f32)
    nmask_d = dram("nmask", [128, WIN], bf16)
    iota_d = dram("iota", [128, 128], bf16)
    ecb_d = dram("ecb", [128, 4], f32)
    wgcn_d = dram("wgcn", [128, 128], bf16)
    bgcnb_d = dram("bgcnb", [128, 128], f32)
    w2_d = dram("w2", [128, 128], bf16)
    b2_d = dram("b2", [128, 1], f32)
    w3_d = dram("w3", [128, A], bf16)
    b3b_d = dram("b3b", [64, A], f32)
    ident_d = dram("ident", [64, 64], bf16)
    out_d = nc.dram_tensor("out", [64, A], f32, kind="ExternalOutput")

    tabsh_d = nc.dram_tensor("tabsh", [SH, 128], bf16)
    tabA_d = nc.dram_tensor("tabA", [NTOTH, 128], bf16, addr_space="Shared")
    tabB_d = nc.dram_tensor("tabB", [NTOTH, 128], bf16, addr_space="Shared")
    pool_in_d = nc.dram_tensor("pool_in", [64, 129], f32)
    pool_out_d = nc.dram_tensor("pool_out", [64, 129], f32, addr_space="Shared")

    groups = [list(range(NC))]
    sq_fn = getattr(Act, "Square")

    with tile.TileContext(nc) as tc:
        with (
            tc.tile_pool(name="const", bufs=1) as cpool,
            tc.tile_pool(name="work", bufs=1) as wpool,
            tc.tile_pool(name="mtile", bufs=13) as mpool,
            tc.tile_pool(name="small", bufs=12) as spool,
            tc.tile_pool(name="hone", bufs=3) as hpool,
            tc.tile_pool(name="psA", bufs=3, space="PSUM") as psA,
            tc.tile_pool(name="psB", bufs=1, space="PSUM") as psB,
            tc.tile_pool(name="psC", bufs=1, space="PSUM") as psC,
        ):
            # ---- constants ----
            iota_t = cpool.tile([128, 128], bf16)
            ec_t = cpool.tile([128, 4], f32)
            bgcn_t = cpool.tile([128, 128], f32)
            nmask_t = cpool.tile([128, WIN], bf16)
            batch_t = cpool.tile([128, WIN], f32)
            wgcn_t = cpool.tile([128, 128], bf16)
            nc.sync.dma_start(out=iota_t[:], in_=iota_d[:])
            nc.sync.dma_start(out=ec_t[:], in_=ecb_d[:])
            nc.sync.dma_start(out=bgcn_t[:], in_=bgcnb_d[:])
            nc.sync.dma_start(out=nmask_t[:], in_=nmask_d[:])
            nc.sync.dma_start(out=batch_t[:], in_=batch_d[:])
            nc.sync.dma_start(out=wgcn_t[:], in_=wgcn_d[:])

            # ---- local xw table (unscaled yet), kept in SBUF ----
            xtall = wpool.tile([128, SH], bf16)
            nc.sync.dma_start(out=xtall[:], in_=xT_d[:])
            locall = wpool.tile([128, SH], bf16)
            loc = [locall[:, w * 128:(w + 1) * 128] for w in range(WIN)]
            for w in range(WIN):
                ps = psA.tile([128, 128], f32, tag="mm")
                nc.tensor.matmul(ps[:], xtall[:, w * 128:(w + 1) * 128], wgcn_t[:],
                                 start=True, stop=True)
                nc.scalar.activation(out=loc[w], in_=ps[:], func=Act.Copy)

            # ---- degree pass: edge MLP + deg + dis ----
            with tc.tile_pool(name="p1", bufs=1) as p1:
                p1s0 = p1.tile([128, R1TOT], f32)
                p1at = p1.tile([128, R1TOT], bf16)
                p1mk = p1.tile([128, R1TOT], bf16)
                nc.sync.dma_start(out=p1s0[:], in_=p1_s0_d[:])
                nc.sync.dma_start(out=p1at[:], in_=p1_attr_d[:])
                nc.sync.dma_start(out=p1mk[:], in_=p1_mask_d[:])
                h_t = p1.tile([128, R1TOT], f32)
                nc.vector.scalar_tensor_tensor(out=h_t[:], in0=p1at[:],
                                               scalar=ec_t[:, 0:1], in1=p1s0[:],
                                               op0=Alu.mult, op1=Alu.add)
                nc.scalar.activation(out=h_t[:], in_=h_t[:], func=Act.Relu)
                wp = p1.tile([128, R1TOT], f32)
                nc.scalar.activation(out=wp[:], in_=h_t[:], func=Act.Sigmoid,
                                     bias=ec_t[:, 2:3], scale=ec_t[:, 1:2])
                nc.vector.tensor_tensor(out=wp[:], in0=wp[:], in1=p1mk[:],
                                        op=Alu.mult)
                deg_t = wpool.tile([128, WIN], f32)
                for w in range(WIN):
                    if R1_w[w] > 0:
                        nc.vector.tensor_reduce(
                            out=deg_t[:, w:w + 1],
                            in_=wp[:, woff1[w]:woff1[w] + R1_w[w]],
                            axis=mybir.AxisListType.X, op=Alu.add)
                    else:
                        nc.vector.memset(deg_t[:, w:w + 1], 0.0)
                nc.vector.tensor_scalar(out=deg_t[:], in0=deg_t[:], scalar1=1.0,
                                        scalar2=None, op0=Alu.add)
                sq_t = wpool.tile([128, WIN], f32)
                nc.scalar.activation(out=sq_t[:], in_=deg_t[:], func=Act.Sqrt)
                dis_t = wpool.tile([128, WIN], f32)
                nc.vector.reciprocal(out=dis_t[:], in_=sq_t[:])

            # ---- scale table by dis, write out, AllGather halves ----
            def wr_tab(w0, w1):
                nw = w1 - w0
                nc.sync.dma_start(
                    out=tabsh_d[w0 * 128:w1 * 128, :].rearrange(
                        "(w p) f -> p w f", p=128),
                    in_=locall[:, w0 * 128:w1 * 128].rearrange(
                        "p (w f) -> p w f", f=128))
            WHALF = (SHH + 127) // 128                # windows covering half A
            for w in range(WIN):
                nc.vector.tensor_scalar(out=loc[w], in0=loc[w],
                                        scalar1=dis_t[:, w:w + 1], scalar2=None,
                                        op0=Alu.mult)
                if w == WHALF - 1:
                    for a in range(0, WHALF, 7):
                        wr_tab(a, min(a + 7, WHALF))
                    nc.gpsimd.collective_compute(
                        "AllGather", Alu.bypass, replica_groups=groups,
                        ins=[tabsh_d[0:SHH, :]], outs=[tabA_d[:]])
            for a in range(WHALF, WIN, 7):
                wr_tab(a, min(a + 7, WIN))
            nc.gpsimd.collective_compute(
                "AllGather", Alu.bypass, replica_groups=groups,
                ins=[tabsh_d[SHH:SH, :]], outs=[tabB_d[:]])

            # ---- block-layout MLP ----
            p2dl = wpool.tile([128, NBLK], f32)
            nc.sync.dma_start(out=p2dl[:], in_=p2_dl_d[:])
            idx_t = wpool.tile([128, NBLK * 8], i16)
            nc.sync.dma_start(out=idx_t[:], in_=p2_idx_d[:])
            w2s = wpool.tile([128, NBLK], f32)
            negw = wpool.tile([128, NBLK], f32)
            ndl = wpool.tile([128, NBLK], f32)
            with tc.tile_pool(name="p2", bufs=1) as p2:
                p2s0 = p2.tile([128, NBLK], f32)
                p2at = p2.tile([128, NBLK], bf16)
                nc.sync.dma_start(out=p2s0[:], in_=p2_s0_d[:])
                nc.sync.dma_start(out=p2at[:], in_=p2_attr_d[:])
                h2t = p2.tile([128, NBLK], f32)
                nc.vector.scalar_tensor_tensor(out=h2t[:], in0=p2at[:],
                                               scalar=ec_t[:, 0:1], in1=p2s0[:],
                                               op0=Alu.mult, op1=Alu.add)
                nc.scalar.activation(out=h2t[:], in_=h2t[:], func=Act.Relu)
                nc.scalar.activation(out=w2s[:], in_=h2t[:], func=Act.Sigmoid,
                                     bias=ec_t[:, 2:3], scale=ec_t[:, 1:2])
            nc.vector.tensor_scalar(out=negw[:], in0=w2s[:], scalar1=-1.0,
                                    scalar2=None, op0=Alu.mult)
            nc.vector.tensor_scalar(out=ndl[:], in0=p2dl[:], scalar1=-1.0,
                                    scalar2=None, op0=Alu.mult)

            # ---- spine: gathers on 4 queues ----
            blk_tile = {}
            for ci, (h, b0, nb) in enumerate(calls):
                mt = mpool.tile([128, nb, 128], bf16, tag="M")
                tab = tabA_d if h == 0 else tabB_d
                nc.gpsimd.dma_gather(
                    out_ap=mt[:],
                    in_ap=tab[:],
                    idxs_ap=idx_t[:, b0 * 8:(b0 + nb) * 8],
                    num_idxs=nb * 128,
                    num_idxs_reg=nb * 128,
                    elem_size=128,
                    single_packet=False,
                    queue_num=ci % 4,
                )
                for i in range(nb):
                    blk_tile[b0 + i] = (mt, i)

            # ---- scatter matmuls + finalize per window ----
            pool_ps = psB.tile([64, 129], f32, tag="poolps")
            gb = 0
            for w in range(WIN):
                blks = win_blocks[w]
                psw = psA.tile([128, 128], f32, tag="mm")
                for bi, b in enumerate(blks):
                    mt, i = blk_tile[b]
                    if gb % 4 == 3:
                        ab = spool.tile([128, 128], f32, tag="ab")
                        nc.scalar.activation(out=ab[:], in_=iota_t[:], func=sq_fn,
                                             bias=ndl[:, b:b + 1], scale=1.0)
                        s_t = spool.tile([128, 128], bf16, tag="sA")
                        nc.scalar.activation(out=s_t[:], in_=ab[:], func=Act.Relu,
                                             bias=w2s[:, b:b + 1],
                                             scale=negw[:, b:b + 1])
                    else:
                        s_t = spool.tile([128, 128], bf16, tag="sV")
                        nc.vector.tensor_scalar(
                            out=s_t[:], in0=iota_t[:],
                            scalar1=p2dl[:, b:b + 1], scalar2=w2s[:, b:b + 1],
                            op0=Alu.is_equal, op1=Alu.mult)
                    gb += 1
                    nc.tensor.matmul(psw[:], s_t[:], mt[:, i, :],
                                     start=(bi == 0), stop=(bi == len(blks) - 1))

                t2 = spool.tile([128, 128], f32, tag="t2")
                nc.vector.scalar_tensor_tensor(out=t2[:], in0=loc[w],
                                               scalar=dis_t[:, w:w + 1],
                                               in1=bgcn_t[:], op0=Alu.mult,
                                               op1=Alu.add)
                pre = spool.tile([128, 128], f32, tag="pre")
                nc.vector.scalar_tensor_tensor(out=pre[:], in0=psw[:],
                                               scalar=dis_t[:, w:w + 1],
                                               in1=t2[:], op0=Alu.mult, op1=Alu.add)
                h1 = hpool.tile([128, 129], bf16, tag="h1")
                nc.scalar.activation(out=h1[:, 0:128], in_=pre[:], func=Act.Relu)
                nc.vector.tensor_copy(out=h1[:, 128:129], in_=nmask_t[:, w:w + 1])

                pw = spool.tile([128, 64], bf16, tag="pw")
                nc.vector.tensor_scalar(
                    out=pw[:], in0=iota_t[:, 0:64],
                    scalar1=batch_t[:, w:w + 1], scalar2=None, op0=Alu.is_equal)
                nc.tensor.matmul(pool_ps[:], pw[:], h1[:],
                                 start=(w == 0), stop=(w == WIN - 1))

            # ---- AllReduce pooled ----
            pool_sb = wpool.tile([64, 129], f32)
            nc.vector.tensor_copy(out=pool_sb[:], in_=pool_ps[:])
            nc.sync.dma_start(out=pool_in_d[:], in_=pool_sb[:])
            nc.gpsimd.collective_compute(
                "AllReduce", Alu.add, replica_groups=groups,
                ins=[pool_in_d[:]], outs=[pool_out_d[:]])
            pool2 = wpool.tile([64, 129], f32)
            nc.sync.dma_start(out=pool2[:], in_=pool_out_d[:])

            # ---- head ----
            cntm = wpool.tile([64, 1], f32)
            nc.vector.tensor_scalar(out=cntm[:], in0=pool2[:, 128:129], scalar1=1.0,
                                    scalar2=None, op0=Alu.max)
            rec = wpool.tile([64, 1], f32)
            nc.vector.reciprocal(out=rec[:], in_=cntm[:])
            pooled_b = wpool.tile([64, 128], bf16)
            nc.vector.tensor_scalar(out=pooled_b[:], in0=pool2[:, 0:128],
                                    scalar1=rec[:], scalar2=None, op0=Alu.mult)

            ident_t = cpool.tile([64, 64], bf16)
            nc.sync.dma_start(out=ident_t[:], in_=ident_d[:])
            psT = psC.tile([128, 64], bf16, tag="pT")
            nc.tensor.transpose(psT[:], pooled_b[:], ident_t[:])
            pooledT = wpool.tile([128, 64], bf16)
            nc.vector.tensor_copy(out=pooledT[:], in_=psT[:])

            w2b = cpool.tile([128, 128], bf16)
            nc.sync.dma_start(out=w2b[:], in_=w2_d[:])
            b2_t = cpool.tile([128, 1], f32)
            nc.sync.dma_start(out=b2_t[:], in_=b2_d[:])
            h2ps = psC.tile([128, 64], f32, tag="h2")
            nc.tensor.matmul(h2ps[:], w2b[:], pooledT[:], start=True, stop=True)
            h2sb = wpool.tile([128, 64], bf16)
            nc.scalar.activation(out=h2sb[:], in_=h2ps[:], func=Act.Relu,
                                 bias=b2_t[:], scale=1.0)

            w3b = cpool.tile([128, A], bf16)
            nc.sync.dma_start(out=w3b[:], in_=w3_d[:])
            b3_t = cpool.tile([64, A], f32)
            nc.sync.dma_start(out=b3_t[:], in_=b3b_d[:])
            yps = psC.tile([64, A], f32, tag="y")
            nc.tensor.matmul(yps[:], h2sb[:], w3b[:], start=True, stop=True)
            ysb = wpool.tile([64, A], f32)
            nc.vector.tensor_tensor(out=ysb[:], in0=yps[:], in1=b3_t[:], op=Alu.add)
            nc.sync.dma_start(out=out_d[:], in_=ysb[:])

    nc.compile()
    return nc


_CACHE = {}


def _get_program(cfg, meta):
    key = (tuple(sorted(cfg.items())), meta["R1TOT"], tuple(meta["R1_w"]),
           meta["NBLK"], tuple(meta["calls"]),
           tuple(tuple(b) for b in meta["win_blocks"]))
    if key not in _CACHE:
        _CACHE[key] = _build(cfg, meta)
    return _CACHE[key]


def kernel(**inputs):
    from concourse import bass_utils
    cfg = _derived(_default_cfg())
    inputs = {k: np.asarray(v) for k, v in inputs.items()}
    in_maps, meta = _prep(cfg, **inputs)
    nc = _get_program(cfg, meta)
    res = bass_utils.run_bass_kernel_spmd(nc, in_maps, list(range(cfg["NCORES"])))
    return np.asarray(res.results[0]["out"], np.float32)[: cfg["G"]]


# revision 13
# speedup vs baseline: 1.1433x; 1.1433x over previous
"""Trainium2 Bass kernel for a GCN-based DQN forward pass (8 NeuronCores).

v2 strategy (dst-sharded nodes+edges, one-hot scatter matmuls):
 - host folds W_e1/b_e1 into a single f32 stream s0 = a*src + b*dst + d
 - unified degree pass (slot layout) -> deg -> dis = 1/sqrt(deg+1)
 - local table shard: dis * (x @ W_gcn) in bf16, kept in SBUF for the
   self-loop term; written to DRAM and AllGathered in TWO rank-halves so
   gathers can start after the first collective
 - spine: dma_gather of per-edge source rows on 4 SWDGE queues
   (round-robin) -> one-hot scatter matmuls per 128-edge block, one-hot
   built 3:1 on DVE (is_eq+mult) / ACT (square + relu trick); padding
   edges carry dstloc=-1 so their one-hot column is zero
 - finalize per window: dis_dst scaling + bias + relu, pooling matmuls,
   AllReduce of pooled sums/counts, replicated tiny MLP head
"""
import numpy as np
import ml_dtypes

BF16 = ml_dtypes.bfloat16


def _default_cfg():
    return dict(N=50000, E=1600000, G=64, A=8, NCORES=8, WIN=49, GRP=7)


def _derived(cfg):
    c = dict(cfg)
    c["SH_REAL"] = -(-c["N"] // c["NCORES"])          # real nodes per core (ceil)
    c["SH"] = c["WIN"] * 128                          # padded nodes per core
    assert c["SH"] >= c["SH_REAL"]
    assert c["SH"] % 2 == 0
    c["SHH"] = c["SH"] // 2                           # rank-half size
    c["NTOTH"] = c["NCORES"] * c["SHH"]               # rows per half table
    assert c["NTOTH"] - 1 <= 32767, "half-table must be int16-indexable"
    c["CALLB"] = 8                                    # blocks per gather call
    return c


def _prep(cfg, x, edge_attr, W_e1, b_e1, W_e2, b_e2, W_gcn, b_gcn, W2, b2, W3, b3,
          edge_index, batch):
    """Host-side sharding/layout. Returns (in_maps, meta)."""
    N, E, G, A = cfg["N"], cfg["E"], cfg["G"], cfg["A"]
    NC, WIN, SH_REAL, SH = cfg["NCORES"], cfg["WIN"], cfg["SH_REAL"], cfg["SH"]
    SHH, CALLB = cfg["SHH"], cfg["CALLB"]

    x = np.asarray(x, np.float32)
    edge_attr = np.asarray(edge_attr, np.float32)
    edge_index = np.asarray(edge_index)
    batch = np.asarray(batch)
    src = np.asarray(edge_index[0], np.int64)
    dst = np.asarray(edge_index[1], np.int64)
    attr = edge_attr[:, 0]

    deg = np.bincount(dst, minlength=N)

    # per-core degree-sorted window/slot assignment
    node_of_rank = np.full((NC, SH), -1, np.int64)   # rank -> orig node id (-1 pad)
    rank_of_orig = np.empty(N, np.int64)             # orig -> rank within its core
    R1_cw = np.zeros((NC, WIN), np.int64)
    for c in range(NC):
        lo, hi = c * SH_REAL, min((c + 1) * SH_REAL, N)
        nreal = hi - lo
        d_loc = np.full(SH, -1, np.int64)
        d_loc[:nreal] = deg[lo:hi]
        order = np.argsort(-d_loc, kind="stable")    # rank -> padded-loc
        rank = np.empty(SH, np.int64)
        rank[order] = np.arange(SH)
        node_of_rank[c] = np.where(order < nreal, lo + order, -1)
        rank_of_orig[lo:hi] = rank[:nreal]
        R1_cw[c] = np.maximum(d_loc[order].reshape(WIN, 128), 0).max(axis=1)

    R1_w = R1_cw.max(axis=0)
    R1TOT = max(int(R1_w.sum()), 1)
    woff1 = np.zeros(WIN + 1, np.int64)
    woff1[1:] = np.cumsum(R1_w)

    core_of = np.minimum(np.arange(N) // SH_REAL, NC - 1)

    # per-edge coordinates
    ecore = np.minimum(dst // SH_REAL, NC - 1)
    erank = rank_of_orig[dst]
    ew = erank // 128
    ep = erank % 128
    score = np.minimum(src // SH_REAL, NC - 1)
    srank = rank_of_orig[src]
    ehalf = (srank >= SHH).astype(np.int64)
    erow = score * SHH + srank - ehalf * SHH         # row within half table

    # j1 = rank of edge within its dst-node's list (degree pass)
    eorder = np.argsort(dst, kind="stable")
    starts = np.zeros(N + 1, np.int64)
    starts[1:] = np.cumsum(deg)
    j1 = np.empty(E, np.int64)
    j1[eorder] = np.arange(E) - starts[dst[eorder]]

    # pass-2 segment = (window, half); per-core counts -> uniform block counts
    segid = ew * 2 + ehalf                            # 0..2*WIN-1
    cnt = np.zeros((NC, 2 * WIN), np.int64)
    for c in range(NC):
        m = ecore == c
        cnt[c] = np.bincount(segid[m], minlength=2 * WIN)
    NB_seg = -(-cnt.max(axis=0) // 128)               # blocks per segment (uniform)

    seg_boff = np.zeros(2 * WIN, np.int64)
    calls = []                                        # (half, block_start, nblocks)
    pos = 0
    for w in range(WIN):
        for h in (0, 1):
            seg_boff[w * 2 + h] = pos
            nseg = int(NB_seg[w * 2 + h])
            b = pos
            pos += nseg
            while b < pos:
                nb = min(CALLB, pos - b)
                calls.append((h, int(b), int(nb)))
                b += nb
    NBLK = max(int(pos), 1)

    # per-window block list in call order
    win_blocks = []
    for w in range(WIN):
        blks = []
        for h in (0, 1):
            b0 = int(seg_boff[w * 2 + h])
            blks.extend(range(b0, b0 + int(NB_seg[w * 2 + h])))
        win_blocks.append(blks)

    # j2 = rank of edge within its (core, segment) group
    keys = (ecore * (2 * WIN) + segid)
    eorder2 = np.argsort(keys, kind="stable")
    gcnt = np.bincount(keys, minlength=NC * 2 * WIN)
    gstarts = np.zeros(NC * 2 * WIN + 1, np.int64)
    gstarts[1:] = np.cumsum(gcnt)
    j2 = np.empty(E, np.int64)
    j2[eorder2] = np.arange(E) - gstarts[keys[eorder2]]

    we1 = np.asarray(W_e1, np.float64).reshape(3)
    be1 = float(np.asarray(b_e1, np.float64).reshape(-1)[0])
    we2 = float(np.asarray(W_e2, np.float64).reshape(-1)[0])
    be2 = float(np.asarray(b_e2, np.float64).reshape(-1)[0])
    s0_all = (we1[0] * src + we1[1] * dst + be1).astype(np.float32)

    ecv = np.array([we1[2], we2, be2, 0.0], np.float32)
    ec_bcast = np.ascontiguousarray(np.broadcast_to(ecv, (128, 4)))

    iota128 = np.ascontiguousarray(
        np.broadcast_to(np.arange(128, dtype=np.float32), (128, 128)).astype(BF16))
    bgcn_b = np.ascontiguousarray(
        np.broadcast_to(np.asarray(b_gcn, np.float32), (128, 128)))
    b3_b = np.ascontiguousarray(
        np.broadcast_to(np.asarray(b3, np.float32), (64, A)))
    ident64 = np.eye(64, dtype=BF16)
    wgcn_b16 = np.ascontiguousarray(np.asarray(W_gcn, np.float32)).astype(BF16)
    w2_b16 = np.ascontiguousarray(np.asarray(W2, np.float32)).astype(BF16)
    w3_b16 = np.ascontiguousarray(np.asarray(W3, np.float32)).astype(BF16)
    b2_np = np.ascontiguousarray(np.asarray(b2, np.float32).reshape(128, 1))

    in_maps = []
    for c in range(NC):
        m = ecore == c
        s_s0, s_attr = s0_all[m], attr[m]
        s_ep, s_ew, s_j1, s_j2 = ep[m], ew[m], j1[m], j2[m]
        s_seg, s_row = segid[m], erow[m]

        # degree-pass slot-layout streams [128, R1TOT]
        p1_s0 = np.zeros((128, R1TOT), np.float32)
        p1_attr = np.zeros((128, R1TOT), BF16)
        p1_mask = np.zeros((128, R1TOT), BF16)
        col1 = woff1[s_ew] + s_j1
        p1_s0[s_ep, col1] = s_s0
        p1_attr[s_ep, col1] = s_attr
        p1_mask[s_ep, col1] = 1.0

        # block-layout streams [128, NBLK]
        p2_s0 = np.zeros((128, NBLK), np.float32)
        p2_attr = np.zeros((128, NBLK), BF16)
        p2_dl = np.full((128, NBLK), -1.0, np.float32)  # pads: one-hot column dead
        blk = seg_boff[s_seg] + s_j2 // 128
        pp = s_j2 % 128
        p2_s0[pp, blk] = s_s0
        p2_attr[pp, blk] = s_attr
        p2_dl[pp, blk] = s_ep

        # gather idx stream, wrapped int16 [128, NBLK*8]; pads fetch row 0
        idx_flat = np.zeros(NBLK * 128, np.int64)
        k = blk * 128 + pp
        idx_flat[k] = s_row
        idx16 = np.zeros((128, NBLK * 8), np.int16)
        wrap = idx_flat.reshape(NBLK * 8, 16).T.astype(np.int16)
        for gg in range(8):
            idx16[gg * 16:(gg + 1) * 16, :] = wrap

        # xT in slot order [128, SH] bf16
        nr = node_of_rank[c]
        valid = nr >= 0
        xs = np.zeros((SH, x.shape[1]), np.float32)
        xs[valid] = x[nr[valid]]
        xT = np.ascontiguousarray(xs.T).astype(BF16)

        batch_slot = np.full((128, WIN), 127.0, np.float32)
        nmask = np.zeros((128, WIN), BF16)
        bvals = np.full(SH, 127, np.int64)
        bvals[valid] = batch[nr[valid]]
        batch_slot[:, :] = bvals.reshape(WIN, 128).T
        nmask[:, :] = (valid.reshape(WIN, 128).T).astype(BF16)

        in_maps.append({
            "p1_s0": p1_s0, "p1_attr": p1_attr, "p1_mask": p1_mask,
            "p2_s0": p2_s0, "p2_attr": p2_attr, "p2_dl": p2_dl, "p2_idx": idx16,
            "xT": xT, "batch_slot": batch_slot, "nmask": nmask,
            "iota": iota128, "ecb": ec_bcast, "wgcn": wgcn_b16, "bgcnb": bgcn_b,
            "w2": w2_b16, "b2": b2_np, "w3": w3_b16, "b3b": b3_b, "ident": ident64,
        })

    meta = dict(R1TOT=R1TOT, R1_w=[int(v) for v in R1_w],
                woff1=[int(v) for v in woff1],
                NBLK=NBLK, calls=calls, win_blocks=win_blocks)
    return in_maps, meta


def _build(cfg, meta):
    from concourse import bass, bacc, tile
    import concourse.mybir as mybir

    f32 = mybir.dt.float32
    bf16 = mybir.dt.bfloat16
    i16 = mybir.dt.int16
    Alu = mybir.AluOpType
    Act = mybir.ActivationFunctionType

    NC, WIN, SH, SHH = cfg["NCORES"], cfg["WIN"], cfg["SH"], cfg["SHH"]
    NTOTH, G, A = cfg["NTOTH"], cfg["G"], cfg["A"]
    R1TOT, R1_w, woff1 = meta["R1TOT"], meta["R1_w"], meta["woff1"]
    NBLK, calls, win_blocks = meta["NBLK"], meta["calls"], meta["win_blocks"]

    nc = bacc.Bacc("TRN2", target_bir_lowering=False, debug=False, num_devices=NC,
                   num_swdge_queues=4)

    dram = lambda nm, shp, dt: nc.dram_tensor(nm, shp, dt, kind="ExternalInput")
    p1_s0_d = dram("p1_s0", [128, R1TOT], f32)
    p1_attr_d = dram("p1_attr", [128, R1TOT], bf16)
    p1_mask_d = dram("p1_mask", [128, R1TOT], bf16)
    p2_s0_d = dram("p2_s0", [128, NBLK], f32)
    p2_attr_d = dram("p2_attr", [128, NBLK], bf16)
    p2_dl_d = dram("p2_dl", [128, NBLK], f32)
    p2_idx_d = dram("p2_idx", [128, NBLK * 8], i16)
    xT_d = dram("xT", [128, SH], bf16)
    batch_d = dram("batch_slot", [128, WIN], f32)
    nmask_d = dram("nmask", [128, WIN], bf16)
    iota_d = dram("iota", [128, 128], bf16)
    ecb_d = dram("ecb", [128, 4], f32)
    wgcn_d = dram("wgcn", [128, 128], bf16)
    bgcnb_d = dram("bgcnb", [128, 128], f32)
    w2_d = dram("w2", [128, 128], bf16)
    b2_d = dram("b2", [128, 1], f32)
    w3_d = dram("w3", [128, A], bf16)
    b3b_d = dram("b3b", [64, A], f32)
    ident_d = dram("ident", [64, 64], bf16)
    out_d = nc.dram_tensor("out", [64, A], f32, kind="ExternalOutput")

    tabsh_d = nc.dram_tensor("tabsh", [SH, 128], bf16)
    tabA_d = nc.dram_tensor("tabA", [NTOTH, 128], bf16, addr_space="Shared")
    tabB_d = nc.dram_tensor("tabB", [NTOTH, 128], bf16, addr_space="Shared")
    pool_in_d = nc.dram_tensor("pool_in", [64, 129], f32)
    pool_out_d = nc.dram_tensor("pool_out", [64, 129], f32, addr_space="Shared")

    groups = [list(range(NC))]
    sq_fn = getattr(Act, "Square")

    with tile.TileContext(nc) as tc:
        with (
            tc.tile_pool(name="const", bufs=1) as cpool,
            tc.tile_pool(name="work", bufs=1) as wpool,
            tc.tile_pool(name="mtile", bufs=40) as mpool,
            tc.tile_pool(name="small", bufs=8) as spool,
            tc.tile_pool(name="hone", bufs=3) as hpool,
            tc.tile_pool(name="psA", bufs=3, space="PSUM") as psA,
            tc.tile_pool(name="psB", bufs=1, space="PSUM") as psB,
            tc.tile_pool(name="psC", bufs=1, space="PSUM") as psC,
        ):
            # ---- constants ----
            iota_t = cpool.tile([128, 128], bf16)
            ec_t = cpool.tile([128, 4], f32)
            bgcn_t = cpool.tile([128, 128], f32)
            nmask_t = cpool.tile([128, WIN], bf16)
            batch_t = cpool.tile([128, WIN], f32)
            wgcn_t = cpool.tile([128, 128], bf16)
            nc.sync.dma_start(out=iota_t[:], in_=iota_d[:])
            nc.sync.dma_start(out=ec_t[:], in_=ecb_d[:])
            nc.sync.dma_start(out=bgcn_t[:], in_=bgcnb_d[:])
            nc.sync.dma_start(out=nmask_t[:], in_=nmask_d[:])
            nc.sync.dma_start(out=batch_t[:], in_=batch_d[:])
            nc.sync.dma_start(out=wgcn_t[:], in_=wgcn_d[:])

            # ---- local xw table (unscaled yet), kept in SBUF ----
            locall = wpool.tile([128, SH], bf16)
            loc = [locall[:, w * 128:(w + 1) * 128] for w in range(WIN)]
            with tc.tile_pool(name="xt", bufs=1) as xpool:
                xtall = xpool.tile([128, SH], bf16)
                nc.sync.dma_start(out=xtall[:], in_=xT_d[:])
                for w in range(WIN):
                    ps = psA.tile([128, 128], f32, tag="mm")
                    nc.tensor.matmul(ps[:], xtall[:, w * 128:(w + 1) * 128],
                                     wgcn_t[:], start=True, stop=True)
                    nc.scalar.activation(out=loc[w], in_=ps[:], func=Act.Copy)

            # ---- degree pass: edge MLP + deg + dis ----
            with tc.tile_pool(name="p1", bufs=1) as p1:
                p1s0 = p1.tile([128, R1TOT], f32)
                p1at = p1.tile([128, R1TOT], bf16)
                p1mk = p1.tile([128, R1TOT], bf16)
                nc.sync.dma_start(out=p1s0[:], in_=p1_s0_d[:])
                nc.sync.dma_start(out=p1at[:], in_=p1_attr_d[:])
                nc.sync.dma_start(out=p1mk[:], in_=p1_mask_d[:])
                h_t = p1.tile([128, R1TOT], f32)
                nc.vector.scalar_tensor_tensor(out=h_t[:], in0=p1at[:],
                                               scalar=ec_t[:, 0:1], in1=p1s0[:],
                                               op0=Alu.mult, op1=Alu.add)
                nc.scalar.activation(out=h_t[:], in_=h_t[:], func=Act.Relu)
                wp = p1.tile([128, R1TOT], f32)
                nc.scalar.activation(out=wp[:], in_=h_t[:], func=Act.Sigmoid,
                                     bias=ec_t[:, 2:3], scale=ec_t[:, 1:2])
                nc.vector.tensor_tensor(out=wp[:], in0=wp[:], in1=p1mk[:],
                                        op=Alu.mult)
                deg_t = wpool.tile([128, WIN], f32)
                for w in range(WIN):
                    if R1_w[w] > 0:
                        nc.vector.tensor_reduce(
                            out=deg_t[:, w:w + 1],
                            in_=wp[:, woff1[w]:woff1[w] + R1_w[w]],
                            axis=mybir.AxisListType.X, op=Alu.add)
                    else:
                        nc.vector.memset(deg_t[:, w:w + 1], 0.0)
                nc.vector.tensor_scalar(out=deg_t[:], in0=deg_t[:], scalar1=1.0,
                                        scalar2=None, op0=Alu.add)
                sq_t = wpool.tile([128, WIN], f32)
                nc.scalar.activation(out=sq_t[:], in_=deg_t[:], func=Act.Sqrt)
                dis_t = wpool.tile([128, WIN], f32)
                nc.vector.reciprocal(out=dis_t[:], in_=sq_t[:])

            # ---- scale table by dis, write out, AllGather halves ----
            def wr_tab(w0, w1):
                nw = w1 - w0
                nc.sync.dma_start(
                    out=tabsh_d[w0 * 128:w1 * 128, :].rearrange(
                        "(w p) f -> p w f", p=128),
                    in_=locall[:, w0 * 128:w1 * 128].rearrange(
                        "p (w f) -> p w f", f=128))
            WHALF = (SHH + 127) // 128                # windows covering half A
            for w in range(WIN):
                nc.vector.tensor_scalar(out=loc[w], in0=loc[w],
                                        scalar1=dis_t[:, w:w + 1], scalar2=None,
                                        op0=Alu.mult)
                if w == WHALF - 1:
                    for a in range(0, WHALF, 7):
                        wr_tab(a, min(a + 7, WHALF))
                    nc.gpsimd.collective_compute(
                        "AllGather", Alu.bypass, replica_groups=groups,
                        ins=[tabsh_d[0:SHH, :]], outs=[tabA_d[:]])
            for a in range(WHALF, WIN, 7):
                wr_tab(a, min(a + 7, WIN))
            nc.gpsimd.collective_compute(
                "AllGather", Alu.bypass, replica_groups=groups,
                ins=[tabsh_d[SHH:SH, :]], outs=[tabB_d[:]])

            # ---- block-layout MLP ----
            p2dl = wpool.tile([128, NBLK], f32)
            nc.sync.dma_start(out=p2dl[:], in_=p2_dl_d[:])
            idx_t = wpool.tile([128, NBLK * 8], i16)
            nc.sync.dma_start(out=idx_t[:], in_=p2_idx_d[:])
            w2s = wpool.tile([128, NBLK], f32)
            negw = wpool.tile([128, NBLK], f32)
            ndl = wpool.tile([128, NBLK], f32)
            with tc.tile_pool(name="p2", bufs=1) as p2:
                p2s0 = p2.tile([128, NBLK], f32)
                p2at = p2.tile([128, NBLK], bf16)
                nc.sync.dma_start(out=p2s0[:], in_=p2_s0_d[:])
                nc.sync.dma_start(out=p2at[:], in_=p2_attr_d[:])
                h2t = p2.tile([128, NBLK], f32)
                nc.vector.scalar_tensor_tensor(out=h2t[:], in0=p2at[:],
                                               scalar=ec_t[:, 0:1], in1=p2s0[:],
                                               op0=Alu.mult, op1=Alu.add)
                nc.scalar.activation(out=h2t[:], in_=h2t[:], func=Act.Relu)
                nc.scalar.activation(out=w2s[:], in_=h2t[:], func=Act.Sigmoid,
                                     bias=ec_t[:, 2:3], scale=ec_t[:, 1:2])
            nc.vector.tensor_scalar(out=negw[:], in0=w2s[:], scalar1=-1.0,
                                    scalar2=None, op0=Alu.mult)
            nc.vector.tensor_scalar(out=ndl[:], in0=p2dl[:], scalar1=-1.0,
                                    scalar2=None, op0=Alu.mult)

            # ---- spine: gathers on 4 queues ----
            blk_tile = {}
            for ci, (h, b0, nb) in enumerate(calls):
                mt = mpool.tile([128, nb, 128], bf16, tag="M")
                tab = tabA_d if h == 0 else tabB_d
                nc.gpsimd.dma_gather(
                    out_ap=mt[:],
                    in_ap=tab[:],
                    idxs_ap=idx_t[:, b0 * 8:(b0 + nb) * 8],
                    num_idxs=nb * 128,
                    num_idxs_reg=nb * 128,
                    elem_size=128,
                    single_packet=False,
                    queue_num=ci % 4,
                )
                for i in range(nb):
                    blk_tile[b0 + i] = (mt, i)

            # ---- scatter matmuls + finalize per window ----
            pool_ps = psB.tile([64, 129], f32, tag="poolps")
            gb = 0
            for w in range(WIN):
                blks = win_blocks[w]
                psw = psA.tile([128, 128], f32, tag="mm")
                for bi, b in enumerate(blks):
                    mt, i = blk_tile[b]
                    if gb % 4 == 3:
                        ab = spool.tile([128, 128], f32, tag="ab")
                        nc.scalar.activation(out=ab[:], in_=iota_t[:], func=sq_fn,
                                             bias=ndl[:, b:b + 1], scale=1.0)
                        s_t = spool.tile([128, 128], bf16, tag="sA")
                        nc.scalar.activation(out=s_t[:], in_=ab[:], func=Act.Relu,
                                             bias=w2s[:, b:b + 1],
                                             scale=negw[:, b:b + 1])
                    else:
                        s_t = spool.tile([128, 128], bf16, tag="sV")
                        nc.vector.tensor_scalar(
                            out=s_t[:], in0=iota_t[:],
                            scalar1=p2dl[:, b:b + 1], scalar2=w2s[:, b:b + 1],
                            op0=Alu.is_equal, op1=Alu.mult)
                    gb += 1
                    nc.tensor.matmul(psw[:], s_t[:], mt[:, i, :],
                                     start=(bi == 0), stop=(bi == len(blks) - 1))

                t2 = spool.tile([128, 128], f32, tag="t2")
                nc.vector.scalar_tensor_tensor(out=t2[:], in0=loc[w],
                                               scalar=dis_t[:, w:w + 1],
                                               in1=bgcn_t[:], op0=Alu.mult,
                                               op1=Alu.add)
                pre = spool.tile([128, 128], f32, tag="pre")
                nc.vector.scalar_tensor_tensor(out=pre[:], in0=psw[:],
                                               scalar=dis_t[:, w:w + 1],
                                               in1=t2[:], op0=Alu.mult, op1=Alu.add)
                h1 = hpool.tile([128, 129], bf16, tag="h1")
                nc.scalar.activation(out=h1[:, 0:128], in_=pre[:], func=Act.Relu)
                nc.vector.tensor_copy(out=h1[:, 128:129], in_=nmask_t[:, w:w + 1])

                pw = spool.tile([128, 64], bf16, tag="pw")
                nc.vector.tensor_scalar(
                    out=pw[:], in0=iota_t[:, 0:64],
                    scalar1=batch_t[:, w:w + 1], scalar2=None, op0=Alu.is_equal)
                nc.tensor.matmul(pool_ps[:], pw[:], h1[:],
                                 start=(w == 0), stop=(w == WIN - 1))

            # ---- AllReduce pooled ----
            pool_sb = wpool.tile([64, 129], f32)
            nc.vector.tensor_copy(out=pool_sb[:], in_=pool_ps[:])
            nc.sync.dma_start(out=pool_in_d[:], in_=pool_sb[:])
            nc.gpsimd.collective_compute(
                "AllReduce", Alu.add, replica_groups=groups,
                ins=[pool_in_d[:]], outs=[pool_out_d[:]])
            pool2 = wpool.tile([64, 129], f32)
            nc.sync.dma_start(out=pool2[:], in_=pool_out_d[:])

            # ---- head ----
            cntm = wpool.tile([64, 1], f32)
            nc.vector.tensor_scalar(out=cntm[:], in0=pool2[:, 128:129], scalar1=1.0,
                                    scalar2=None, op0=Alu.max)
            rec = wpool.tile([64, 1], f32)
            nc.vector.reciprocal(out=rec[:], in_=cntm[:])
            pooled_b = wpool.tile([64, 128], bf16)
            nc.vector.tensor_scalar(out=pooled_b[:], in0=pool2[:, 0:128],
                                    scalar1=rec[:], scalar2=None, op0=Alu.mult)

            ident_t = cpool.tile([64, 64], bf16)
            nc.sync.dma_start(out=ident_t[:], in_=ident_d[:])
            psT = psC.tile([128, 64], bf16, tag="pT")
            nc.tensor.transpose(psT[:], pooled_b[:], ident_t[:])
            pooledT = wpool.tile([128, 64], bf16)
            nc.vector.tensor_copy(out=pooledT[:], in_=psT[:])

            w2b = cpool.tile([128, 128], bf16)
            nc.sync.dma_start(out=w2b[:], in_=w2_d[:])
            b2_t = cpool.tile([128, 1], f32)
            nc.sync.dma_start(out=b2_t[:], in_=b2_d[:])
            h2ps = psC.tile([128, 64], f32, tag="h2")
            nc.tensor.matmul(h2ps[:], w2b[:], pooledT[:], start=True, stop=True)
            h2sb = wpool.tile([128, 64], bf16)
            nc.scalar.activation(out=h2sb[:], in_=h2ps[:], func=Act.Relu,
                                 bias=b2_t[:], scale=1.0)

            w3b = cpool.tile([128, A], bf16)
            nc.sync.dma_start(out=w3b[:], in_=w3_d[:])
            b3_t = cpool.tile([64, A], f32)
            nc.sync.dma_start(out=b3_t[:], in_=b3b_d[:])
            yps = psC.tile([64, A], f32, tag="y")
            nc.tensor.matmul(yps[:], h2sb[:], w3b[:], start=True, stop=True)
            ysb = wpool.tile([64, A], f32)
            nc.vector.tensor_tensor(out=ysb[:], in0=yps[:], in1=b3_t[:], op=Alu.add)
            nc.sync.dma_start(out=out_d[:], in_=ysb[:])

    nc.compile()
    return nc


_CACHE = {}


def _get_program(cfg, meta):
    key = (tuple(sorted(cfg.items())), meta["R1TOT"], tuple(meta["R1_w"]),
           meta["NBLK"], tuple(meta["calls"]),
           tuple(tuple(b) for b in meta["win_blocks"]))
    if key not in _CACHE:
        _CACHE[key] = _build(cfg, meta)
    return _CACHE[key]


def kernel(**inputs):
    from concourse import bass_utils
    cfg = _derived(_default_cfg())
    inputs = {k: np.asarray(v) for k, v in inputs.items()}
    in_maps, meta = _prep(cfg, **inputs)
    nc = _get_program(cfg, meta)
    res = bass_utils.run_bass_kernel_spmd(nc, in_maps, list(range(cfg["NCORES"])))
    return np.asarray(res.results[0]["out"], np.float32)[: cfg["G"]]


# revision 15
# speedup vs baseline: 1.2102x; 1.0585x over previous
"""Trainium2 Bass kernel for a GCN-based DQN forward pass (8 NeuronCores).

v2 strategy (dst-sharded nodes+edges, one-hot scatter matmuls):
 - host folds W_e1/b_e1 into a single f32 stream s0 = a*src + b*dst + d
 - unified degree pass (slot layout) -> deg -> dis = 1/sqrt(deg+1)
 - local table shard: dis * (x @ W_gcn) in bf16, kept in SBUF for the
   self-loop term; written to DRAM and AllGathered in TWO rank-halves so
   gathers can start after the first collective
 - spine: dma_gather of per-edge source rows on 4 SWDGE queues
   (round-robin) -> one-hot scatter matmuls per 128-edge block, one-hot
   built 3:1 on DVE (is_eq+mult) / ACT (square + relu trick); padding
   edges carry dstloc=-1 so their one-hot column is zero
 - finalize per window: dis_dst scaling + bias + relu, pooling matmuls,
   AllReduce of pooled sums/counts, replicated tiny MLP head
"""
import numpy as np
import ml_dtypes

BF16 = ml_dtypes.bfloat16


def _default_cfg():
    return dict(N=50000, E=1600000, G=64, A=8, NCORES=8, WIN=49, GRP=7)


def _derived(cfg):
    c = dict(cfg)
    c["SH_REAL"] = -(-c["N"] // c["NCORES"])          # real nodes per core (ceil)
    c["SH"] = c["WIN"] * 128                          # padded nodes per core
    assert c["SH"] >= c["SH_REAL"]
    assert c["SH"] % 2 == 0
    c["SHH"] = c["SH"] // 2                           # rank-half size
    c["NTOTH"] = c["NCORES"] * c["SHH"]               # rows per half table
    assert c["NTOTH"] - 1 <= 32767, "half-table must be int16-indexable"
    c["CALLB"] = 8                                    # blocks per gather call
    return c


def _prep(cfg, x, edge_attr, W_e1, b_e1, W_e2, b_e2, W_gcn, b_gcn, W2, b2, W3, b3,
          edge_index, batch):
    """Host-side sharding/layout. Returns (in_maps, meta)."""
    N, E, G, A = cfg["N"], cfg["E"], cfg["G"], cfg["A"]
    NC, WIN, SH_REAL, SH = cfg["NCORES"], cfg["WIN"], cfg["SH_REAL"], cfg["SH"]
    SHH, CALLB = cfg["SHH"], cfg["CALLB"]

    x = np.asarray(x, np.float32)
    edge_attr = np.asarray(edge_attr, np.float32)
    edge_index = np.asarray(edge_index)
    batch = np.asarray(batch)
    src = np.asarray(edge_index[0], np.int64)
    dst = np.asarray(edge_index[1], np.int64)
    attr = edge_attr[:, 0]

    deg = np.bincount(dst, minlength=N)

    # per-core degree-sorted window/slot assignment
    node_of_rank = np.full((NC, SH), -1, np.int64)   # rank -> orig node id (-1 pad)
    rank_of_orig = np.empty(N, np.int64)             # orig -> rank within its core
    R1_cw = np.zeros((NC, WIN), np.int64)
    for c in range(NC):
        lo, hi = c * SH_REAL, min((c + 1) * SH_REAL, N)
        nreal = hi - lo
        d_loc = np.full(SH, -1, np.int64)
        d_loc[:nreal] = deg[lo:hi]
        order = np.argsort(-d_loc, kind="stable")    # rank -> padded-loc
        rank = np.empty(SH, np.int64)
        rank[order] = np.arange(SH)
        node_of_rank[c] = np.where(order < nreal, lo + order, -1)
        rank_of_orig[lo:hi] = rank[:nreal]
        R1_cw[c] = np.maximum(d_loc[order].reshape(WIN, 128), 0).max(axis=1)

    R1_w = R1_cw.max(axis=0)
    R1TOT = max(int(R1_w.sum()), 1)
    woff1 = np.zeros(WIN + 1, np.int64)
    woff1[1:] = np.cumsum(R1_w)

    core_of = np.minimum(np.arange(N) // SH_REAL, NC - 1)

    # per-edge coordinates
    ecore = np.minimum(dst // SH_REAL, NC - 1)
    erank = rank_of_orig[dst]
    ew = erank // 128
    ep = erank % 128
    score = np.minimum(src // SH_REAL, NC - 1)
    srank = rank_of_orig[src]
    ehalf = (srank >= SHH).astype(np.int64)
    erow = score * SHH + srank - ehalf * SHH         # row within half table

    # j1 = rank of edge within its dst-node's list (degree pass)
    eorder = np.argsort(dst, kind="stable")
    starts = np.zeros(N + 1, np.int64)
    starts[1:] = np.cumsum(deg)
    j1 = np.empty(E, np.int64)
    j1[eorder] = np.arange(E) - starts[dst[eorder]]

    # pass-2 segment = (window, half); per-core counts -> uniform block counts
    segid = ew * 2 + ehalf                            # 0..2*WIN-1
    cnt = np.zeros((NC, 2 * WIN), np.int64)
    for c in range(NC):
        m = ecore == c
        cnt[c] = np.bincount(segid[m], minlength=2 * WIN)
    NB_seg = -(-cnt.max(axis=0) // 128)               # blocks per segment (uniform)

    seg_boff = np.zeros(2 * WIN, np.int64)
    calls = []                                        # (half, block_start, nblocks)
    pos = 0
    for w in range(WIN):
        for h in (0, 1):
            seg_boff[w * 2 + h] = pos
            nseg = int(NB_seg[w * 2 + h])
            b = pos
            pos += nseg
            while b < pos:
                nb = min(CALLB, pos - b)
                calls.append((h, int(b), int(nb)))
                b += nb
    NBLK = max(int(pos), 1)

    # per-window block list in call order
    win_blocks = []
    for w in range(WIN):
        blks = []
        for h in (0, 1):
            b0 = int(seg_boff[w * 2 + h])
            blks.extend(range(b0, b0 + int(NB_seg[w * 2 + h])))
        win_blocks.append(blks)

    # j2 = rank of edge within its (core, segment) group
    keys = (ecore * (2 * WIN) + segid)
    eorder2 = np.argsort(keys, kind="stable")
    gcnt = np.bincount(keys, minlength=NC * 2 * WIN)
    gstarts = np.zeros(NC * 2 * WIN + 1, np.int64)
    gstarts[1:] = np.cumsum(gcnt)
    j2 = np.empty(E, np.int64)
    j2[eorder2] = np.arange(E) - gstarts[keys[eorder2]]

    we1 = np.asarray(W_e1, np.float64).reshape(3)
    be1 = float(np.asarray(b_e1, np.float64).reshape(-1)[0])
    we2 = float(np.asarray(W_e2, np.float64).reshape(-1)[0])
    be2 = float(np.asarray(b_e2, np.float64).reshape(-1)[0])
    s0_all = (we1[0] * src + we1[1] * dst + be1).astype(np.float32)

    ecv = np.array([we1[2], we2, be2, 0.0], np.float32)
    ec_bcast = np.ascontiguousarray(np.broadcast_to(ecv, (128, 4)))

    iota128 = np.ascontiguousarray(
        np.broadcast_to(np.arange(128, dtype=np.float32), (128, 128)).astype(BF16))
    bgcn_b = np.ascontiguousarray(
        np.broadcast_to(np.asarray(b_gcn, np.float32), (128, 128)))
    b3_b = np.ascontiguousarray(
        np.broadcast_to(np.asarray(b3, np.float32), (64, A)))
    ident64 = np.eye(64, dtype=BF16)
    wgcn_b16 = np.ascontiguousarray(np.asarray(W_gcn, np.float32)).astype(BF16)
    w2_b16 = np.ascontiguousarray(np.asarray(W2, np.float32)).astype(BF16)
    w3_b16 = np.ascontiguousarray(np.asarray(W3, np.float32)).astype(BF16)
    b2_np = np.ascontiguousarray(np.asarray(b2, np.float32).reshape(128, 1))

    in_maps = []
    for c in range(NC):
        m = ecore == c
        s_s0, s_attr = s0_all[m], attr[m]
        s_ep, s_ew, s_j1, s_j2 = ep[m], ew[m], j1[m], j2[m]
        s_seg, s_row = segid[m], erow[m]

        # degree-pass slot-layout streams [128, R1TOT]
        p1_s0 = np.zeros((128, R1TOT), np.float32)
        p1_attr = np.zeros((128, R1TOT), BF16)
        p1_mask = np.zeros((128, R1TOT), BF16)
        col1 = woff1[s_ew] + s_j1
        p1_s0[s_ep, col1] = s_s0
        p1_attr[s_ep, col1] = s_attr
        p1_mask[s_ep, col1] = 1.0

        # block-layout streams [128, NBLK]
        p2_s0 = np.zeros((128, NBLK), np.float32)
        p2_attr = np.zeros((128, NBLK), BF16)
        p2_dl = np.full((128, NBLK), -1.0, np.float32)  # pads: one-hot column dead
        blk = seg_boff[s_seg] + s_j2 // 128
        pp = s_j2 % 128
        p2_s0[pp, blk] = s_s0
        p2_attr[pp, blk] = s_attr
        p2_dl[pp, blk] = s_ep

        # gather idx stream, wrapped int16 [128, NBLK*8]; pads fetch row 0
        idx_flat = np.zeros(NBLK * 128, np.int64)
        k = blk * 128 + pp
        idx_flat[k] = s_row
        idx16 = np.zeros((128, NBLK * 8), np.int16)
        wrap = idx_flat.reshape(NBLK * 8, 16).T.astype(np.int16)
        for gg in range(8):
            idx16[gg * 16:(gg + 1) * 16, :] = wrap

        # xT in slot order [128, SH] bf16
        nr = node_of_rank[c]
        valid = nr >= 0
        xs = np.zeros((SH, x.shape[1]), np.float32)
        xs[valid] = x[nr[valid]]
        xT = np.ascontiguousarray(xs.T).astype(BF16)

        batch_slot = np.full((128, WIN), 127.0, np.float32)
        nmask = np.zeros((128, WIN), BF16)
        bvals = np.full(SH, 127, np.int64)
        bvals[valid] = batch[nr[valid]]
        batch_slot[:, :] = bvals.reshape(WIN, 128).T
        nmask[:, :] = (valid.reshape(WIN, 128).T).astype(BF16)

        in_maps.append({
            "p1_s0": p1_s0, "p1_attr": p1_attr, "p1_mask": p1_mask,
            "p2_s0": p2_s0, "p2_attr": p2_attr, "p2_dl": p2_dl, "p2_idx": idx16,
            "xT": xT, "batch_slot": batch_slot, "nmask": nmask,
            "iota": iota128, "ecb": ec_bcast, "wgcn": wgcn_b16, "bgcnb": bgcn_b,
            "w2": w2_b16, "b2": b2_np, "w3": w3_b16, "b3b": b3_b, "ident": ident64,
        })

    meta = dict(R1TOT=R1TOT, R1_w=[int(v) for v in R1_w],
                woff1=[int(v) for v in woff1],
                NBLK=NBLK, calls=calls, win_blocks=win_blocks)
    return in_maps, meta


def _build(cfg, meta):
    from concourse import bass, bacc, tile
    import concourse.mybir as mybir

    f32 = mybir.dt.float32
    bf16 = mybir.dt.bfloat16
    i16 = mybir.dt.int16
    Alu = mybir.AluOpType
    Act = mybir.ActivationFunctionType

    NC, WIN, SH, SHH = cfg["NCORES"], cfg["WIN"], cfg["SH"], cfg["SHH"]
    NTOTH, G, A = cfg["NTOTH"], cfg["G"], cfg["A"]
    R1TOT, R1_w, woff1 = meta["R1TOT"], meta["R1_w"], meta["woff1"]
    NBLK, calls, win_blocks = meta["NBLK"], meta["calls"], meta["win_blocks"]

    nc = bacc.Bacc("TRN2", target_bir_lowering=False, debug=False, num_devices=NC,
                   num_swdge_queues=4)

    dram = lambda nm, shp, dt: nc.dram_tensor(nm, shp, dt, kind="ExternalInput")
    p1_s0_d = dram("p1_s0", [128, R1TOT], f32)
    p1_attr_d = dram("p1_attr", [128, R1TOT], bf16)
    p1_mask_d = dram("p1_mask", [128, R1TOT], bf16)
    p2_s0_d = dram("p2_s0", [128, NBLK], f32)
    p2_attr_d = dram("p2_attr", [128, NBLK], bf16)
    p2_dl_d = dram("p2_dl", [128, NBLK], f32)
    p2_idx_d = dram("p2_idx", [128, NBLK * 8], i16)
    xT_d = dram("xT", [128, SH], bf16)
    batch_d = dram("batch_slot", [128, WIN], f32)
    nmask_d = dram("nmask", [128, WIN], bf16)
    iota_d = dram("iota", [128, 128], bf16)
    ecb_d = dram("ecb", [128, 4], f32)
    wgcn_d = dram("wgcn", [128, 128], bf16)
    bgcnb_d = dram("bgcnb", [128, 128], f32)
    w2_d = dram("w2", [128, 128], bf16)
    b2_d = dram("b2", [128, 1], f32)
    w3_d = dram("w3", [128, A], bf16)
    b3b_d = dram("b3b", [64, A], f32)
    ident_d = dram("ident", [64, 64], bf16)
    out_d = nc.dram_tensor("out", [64, A], f32, kind="ExternalOutput")

    tabsh_d = nc.dram_tensor("tabsh", [SH, 128], bf16)
    tabA_d = nc.dram_tensor("tabA", [NTOTH, 128], bf16, addr_space="Shared")
    tabB_d = nc.dram_tensor("tabB", [NTOTH, 128], bf16, addr_space="Shared")
    pool_in_d = nc.dram_tensor("pool_in", [64, 129], f32)
    pool_out_d = nc.dram_tensor("pool_out", [64, 129], f32, addr_space="Shared")

    groups = [list(range(NC))]
    sq_fn = getattr(Act, "Square")

    with tile.TileContext(nc) as tc:
        with (
            tc.tile_pool(name="const", bufs=1) as cpool,
            tc.tile_pool(name="work", bufs=1) as wpool,
            tc.tile_pool(name="mtile", bufs=28) as mpool,
            tc.tile_pool(name="small", bufs=12) as spool,
            tc.tile_pool(name="hone", bufs=3) as hpool,
            tc.tile_pool(name="psA", bufs=3, space="PSUM") as psA,
            tc.tile_pool(name="psB", bufs=1, space="PSUM") as psB,
            tc.tile_pool(name="psC", bufs=1, space="PSUM") as psC,
        ):
            # ---- constants ----
            iota_t = cpool.tile([128, 128], bf16)
            ec_t = cpool.tile([128, 4], f32)
            bgcn_t = cpool.tile([128, 128], f32)
            nmask_t = cpool.tile([128, WIN], bf16)
            batch_t = cpool.tile([128, WIN], f32)
            wgcn_t = cpool.tile([128, 128], bf16)
            nc.sync.dma_start(out=iota_t[:], in_=iota_d[:])
            nc.sync.dma_start(out=ec_t[:], in_=ecb_d[:])
            nc.sync.dma_start(out=bgcn_t[:], in_=bgcnb_d[:])
            nc.sync.dma_start(out=nmask_t[:], in_=nmask_d[:])
            nc.sync.dma_start(out=batch_t[:], in_=batch_d[:])
            nc.sync.dma_start(out=wgcn_t[:], in_=wgcn_d[:])

            # ---- local xw table (unscaled yet), kept in SBUF ----
            locall = wpool.tile([128, SH], bf16)
            loc = [locall[:, w * 128:(w + 1) * 128] for w in range(WIN)]
            with tc.tile_pool(name="xt", bufs=1) as xpool:
                xtall = xpool.tile([128, SH], bf16)
                nc.sync.dma_start(out=xtall[:], in_=xT_d[:])
                for w in range(WIN):
                    ps = psA.tile([128, 128], f32, tag="mm")
                    nc.tensor.matmul(ps[:], xtall[:, w * 128:(w + 1) * 128],
                                     wgcn_t[:], start=True, stop=True)
                    nc.scalar.activation(out=loc[w], in_=ps[:], func=Act.Copy)

            # ---- degree pass: edge MLP + deg + dis ----
            with tc.tile_pool(name="p1", bufs=1) as p1:
                p1s0 = p1.tile([128, R1TOT], f32)
                p1at = p1.tile([128, R1TOT], bf16)
                p1mk = p1.tile([128, R1TOT], bf16)
                nc.sync.dma_start(out=p1s0[:], in_=p1_s0_d[:])
                nc.sync.dma_start(out=p1at[:], in_=p1_attr_d[:])
                nc.sync.dma_start(out=p1mk[:], in_=p1_mask_d[:])
                h_t = p1.tile([128, R1TOT], f32)
                nc.vector.scalar_tensor_tensor(out=h_t[:], in0=p1at[:],
                                               scalar=ec_t[:, 0:1], in1=p1s0[:],
                                               op0=Alu.mult, op1=Alu.add)
                nc.scalar.activation(out=h_t[:], in_=h_t[:], func=Act.Relu)
                wp = p1.tile([128, R1TOT], f32)
                nc.scalar.activation(out=wp[:], in_=h_t[:], func=Act.Sigmoid,
                                     bias=ec_t[:, 2:3], scale=ec_t[:, 1:2])
                nc.vector.tensor_tensor(out=wp[:], in0=wp[:], in1=p1mk[:],
                                        op=Alu.mult)
                deg_t = wpool.tile([128, WIN], f32)
                for w in range(WIN):
                    if R1_w[w] > 0:
                        nc.vector.tensor_reduce(
                            out=deg_t[:, w:w + 1],
                            in_=wp[:, woff1[w]:woff1[w] + R1_w[w]],
                            axis=mybir.AxisListType.X, op=Alu.add)
                    else:
                        nc.vector.memset(deg_t[:, w:w + 1], 0.0)
                nc.vector.tensor_scalar(out=deg_t[:], in0=deg_t[:], scalar1=1.0,
                                        scalar2=None, op0=Alu.add)
                sq_t = wpool.tile([128, WIN], f32)
                nc.scalar.activation(out=sq_t[:], in_=deg_t[:], func=Act.Sqrt)
                dis_t = wpool.tile([128, WIN], f32)
                nc.vector.reciprocal(out=dis_t[:], in_=sq_t[:])

            # ---- scale table by dis, write out, AllGather halves ----
            def wr_tab(w0, w1):
                nw = w1 - w0
                nc.sync.dma_start(
                    out=tabsh_d[w0 * 128:w1 * 128, :].rearrange(
                        "(w p) f -> p w f", p=128),
                    in_=locall[:, w0 * 128:w1 * 128].rearrange(
                        "p (w f) -> p w f", f=128))
            WHALF = (SHH + 127) // 128                # windows covering half A
            for w in range(WIN):
                nc.vector.tensor_scalar(out=loc[w], in0=loc[w],
                                        scalar1=dis_t[:, w:w + 1], scalar2=None,
                                        op0=Alu.mult)
                if w == WHALF - 1:
                    for a in range(0, WHALF, 7):
                        wr_tab(a, min(a + 7, WHALF))
                    nc.gpsimd.collective_compute(
                        "AllGather", Alu.bypass, replica_groups=groups,
                        ins=[tabsh_d[0:SHH, :]], outs=[tabA_d[:]])
            for a in range(WHALF, WIN, 7):
                wr_tab(a, min(a + 7, WIN))
            nc.gpsimd.collective_compute(
                "AllGather", Alu.bypass, replica_groups=groups,
                ins=[tabsh_d[SHH:SH, :]], outs=[tabB_d[:]])

            # ---- block-layout MLP ----
            p2dl = wpool.tile([128, NBLK], f32)
            nc.sync.dma_start(out=p2dl[:], in_=p2_dl_d[:])
            idx_t = wpool.tile([128, NBLK * 8], i16)
            nc.sync.dma_start(out=idx_t[:], in_=p2_idx_d[:])
            w2s = wpool.tile([128, NBLK], f32)
            negw = wpool.tile([128, NBLK], f32)
            ndl = wpool.tile([128, NBLK], f32)
            with tc.tile_pool(name="p2", bufs=1) as p2:
                p2s0 = p2.tile([128, NBLK], f32)
                p2at = p2.tile([128, NBLK], bf16)
                nc.sync.dma_start(out=p2s0[:], in_=p2_s0_d[:])
                nc.sync.dma_start(out=p2at[:], in_=p2_attr_d[:])
                h2t = p2.tile([128, NBLK], f32)
                nc.vector.scalar_tensor_tensor(out=h2t[:], in0=p2at[:],
                                               scalar=ec_t[:, 0:1], in1=p2s0[:],
                                               op0=Alu.mult, op1=Alu.add)
                nc.scalar.activation(out=h2t[:], in_=h2t[:], func=Act.Relu)
                nc.scalar.activation(out=w2s[:], in_=h2t[:], func=Act.Sigmoid,
                                     bias=ec_t[:, 2:3], scale=ec_t[:, 1:2])
            nc.vector.tensor_scalar(out=negw[:], in0=w2s[:], scalar1=-1.0,
                                    scalar2=None, op0=Alu.mult)
            nc.vector.tensor_scalar(out=ndl[:], in0=p2dl[:], scalar1=-1.0,
                                    scalar2=None, op0=Alu.mult)

            # self-loop + bias term per window, hoisted off the spine
            t2all = wpool.tile([128, SH], bf16)
            for w in range(WIN):
                nc.vector.scalar_tensor_tensor(
                    out=t2all[:, w * 128:(w + 1) * 128], in0=loc[w],
                    scalar=dis_t[:, w:w + 1], in1=bgcn_t[:],
                    op0=Alu.mult, op1=Alu.add)

            # ---- spine: gathers on 4 queues ----
            blk_tile = {}
            for ci, (h, b0, nb) in enumerate(calls):
                mt = mpool.tile([128, nb, 128], bf16, tag="M")
                tab = tabA_d if h == 0 else tabB_d
                nc.gpsimd.dma_gather(
                    out_ap=mt[:],
                    in_ap=tab[:],
                    idxs_ap=idx_t[:, b0 * 8:(b0 + nb) * 8],
                    num_idxs=nb * 128,
                    num_idxs_reg=nb * 128,
                    elem_size=128,
                    single_packet=False,
                    queue_num=ci % 4,
                )
                for i in range(nb):
                    blk_tile[b0 + i] = (mt, i)

            # ---- scatter matmuls + (delayed) finalize per window ----
            pool_ps = psB.tile([64, 129], f32, tag="poolps")

            def emit_fin(w, psw):
                pre = spool.tile([128, 128], f32, tag="pre")
                nc.vector.scalar_tensor_tensor(
                    out=pre[:], in0=psw[:], scalar=dis_t[:, w:w + 1],
                    in1=t2all[:, w * 128:(w + 1) * 128],
                    op0=Alu.mult, op1=Alu.add)
                h1 = hpool.tile([128, 129], bf16, tag="h1")
                nc.scalar.activation(out=h1[:, 0:128], in_=pre[:], func=Act.Relu)
                nc.vector.tensor_copy(out=h1[:, 128:129], in_=nmask_t[:, w:w + 1])
                pw = spool.tile([128, 64], bf16, tag="pw")
                nc.vector.tensor_scalar(
                    out=pw[:], in0=iota_t[:, 0:64],
                    scalar1=batch_t[:, w:w + 1], scalar2=None, op0=Alu.is_equal)
                nc.tensor.matmul(pool_ps[:], pw[:], h1[:],
                                 start=(w == 0), stop=(w == WIN - 1))

            gb = 0
            for w in range(WIN):
                blks = win_blocks[w]
                psw = psA.tile([128, 128], f32, tag="mm")
                for bi, b in enumerate(blks):
                    mt, i = blk_tile[b]
                    if gb % 4 == 3:
                        ab = spool.tile([128, 128], f32, tag="ab")
                        nc.scalar.activation(out=ab[:], in_=iota_t[:], func=sq_fn,
                                             bias=ndl[:, b:b + 1], scale=1.0)
                        s_t = spool.tile([128, 128], bf16, tag="sA")
                        nc.scalar.activation(out=s_t[:], in_=ab[:], func=Act.Relu,
                                             bias=w2s[:, b:b + 1],
                                             scale=negw[:, b:b + 1])
                    else:
                        s_t = spool.tile([128, 128], bf16, tag="sV")
                        nc.vector.tensor_scalar(
                            out=s_t[:], in0=iota_t[:],
                            scalar1=p2dl[:, b:b + 1], scalar2=w2s[:, b:b + 1],
                            op0=Alu.is_equal, op1=Alu.mult)
                    gb += 1
                    nc.tensor.matmul(psw[:], s_t[:], mt[:, i, :],
                                     start=(bi == 0), stop=(bi == len(blks) - 1))
                emit_fin(w, psw)

            # ---- AllReduce pooled ----
            pool_sb = wpool.tile([64, 129], f32)
            nc.vector.tensor_copy(out=pool_sb[:], in_=pool_ps[:])
            nc.sync.dma_start(out=pool_in_d[:], in_=pool_sb[:])
            nc.gpsimd.collective_compute(
                "AllReduce", Alu.add, replica_groups=groups,
                ins=[pool_in_d[:]], outs=[pool_out_d[:]])
            pool2 = wpool.tile([64, 129], f32)
            nc.sync.dma_start(out=pool2[:], in_=pool_out_d[:])

            # ---- head ----
            cntm = wpool.tile([64, 1], f32)
            nc.vector.tensor_scalar(out=cntm[:], in0=pool2[:, 128:129], scalar1=1.0,
                                    scalar2=None, op0=Alu.max)
            rec = wpool.tile([64, 1], f32)
            nc.vector.reciprocal(out=rec[:], in_=cntm[:])
            pooled_b = wpool.tile([64, 128], bf16)
            nc.vector.tensor_scalar(out=pooled_b[:], in0=pool2[:, 0:128],
                                    scalar1=rec[:], scalar2=None, op0=Alu.mult)

            ident_t = cpool.tile([64, 64], bf16)
            nc.sync.dma_start(out=ident_t[:], in_=ident_d[:])
            psT = psC.tile([128, 64], bf16, tag="pT")
            nc.tensor.transpose(psT[:], pooled_b[:], ident_t[:])
            pooledT = wpool.tile([128, 64], bf16)
            nc.vector.tensor_copy(out=pooledT[:], in_=psT[:])

            w2b = cpool.tile([128, 128], bf16)
            nc.sync.dma_start(out=w2b[:], in_=w2_d[:])
            b2_t = cpool.tile([128, 1], f32)
            nc.sync.dma_start(out=b2_t[:], in_=b2_d[:])
            h2ps = psC.tile([128, 64], f32, tag="h2")
            nc.tensor.matmul(h2ps[:], w2b[:], pooledT[:], start=True, stop=True)
            h2sb = wpool.tile([128, 64], bf16)
            nc.scalar.activation(out=h2sb[:], in_=h2ps[:], func=Act.Relu,
                                 bias=b2_t[:], scale=1.0)

            w3b = cpool.tile([128, A], bf16)
            nc.sync.dma_start(out=w3b[:], in_=w3_d[:])
            b3_t = cpool.tile([64, A], f32)
            nc.sync.dma_start(out=b3_t[:], in_=b3b_d[:])
            yps = psC.tile([64, A], f32, tag="y")
            nc.tensor.matmul(yps[:], h2sb[:], w3b[:], start=True, stop=True)
            ysb = wpool.tile([64, A], f32)
            nc.vector.tensor_tensor(out=ysb[:], in0=yps[:], in1=b3_t[:], op=Alu.add)
            nc.sync.dma_start(out=out_d[:], in_=ysb[:])

    nc.compile()
    return nc


_CACHE = {}


def _get_program(cfg, meta):
    key = (tuple(sorted(cfg.items())), meta["R1TOT"], tuple(meta["R1_w"]),
           meta["NBLK"], tuple(meta["calls"]),
           tuple(tuple(b) for b in meta["win_blocks"]))
    if key not in _CACHE:
        _CACHE[key] = _build(cfg, meta)
    return _CACHE[key]


def kernel(**inputs):
    from concourse import bass_utils
    cfg = _derived(_default_cfg())
    inputs = {k: np.asarray(v) for k, v in inputs.items()}
    in_maps, meta = _prep(cfg, **inputs)
    nc = _get_program(cfg, meta)
    res = bass_utils.run_bass_kernel_spmd(nc, in_maps, list(range(cfg["NCORES"])))
    return np.asarray(res.results[0]["out"], np.float32)[: cfg["G"]]


# revision 16
# speedup vs baseline: 1.2446x; 1.0284x over previous
"""Trainium2 Bass kernel for a GCN-based DQN forward pass (8 NeuronCores).

v2 strategy (dst-sharded nodes+edges, one-hot scatter matmuls):
 - host folds W_e1/b_e1 into a single f32 stream s0 = a*src + b*dst + d
 - unified degree pass (slot layout) -> deg -> dis = 1/sqrt(deg+1)
 - local table shard: dis * (x @ W_gcn) in bf16, kept in SBUF for the
   self-loop term; written to DRAM and AllGathered in TWO rank-halves so
   gathers can start after the first collective
 - spine: dma_gather of per-edge source rows on 4 SWDGE queues
   (round-robin) -> one-hot scatter matmuls per 128-edge block, one-hot
   built 3:1 on DVE (is_eq+mult) / ACT (square + relu trick); padding
   edges carry dstloc=-1 so their one-hot column is zero
 - finalize per window: dis_dst scaling + bias + relu, pooling matmuls,
   AllReduce of pooled sums/counts, replicated tiny MLP head
"""
import numpy as np
import ml_dtypes

BF16 = ml_dtypes.bfloat16


def _default_cfg():
    return dict(N=50000, E=1600000, G=64, A=8, NCORES=8, WIN=49, GRP=7)


def _derived(cfg):
    c = dict(cfg)
    c["SH_REAL"] = -(-c["N"] // c["NCORES"])          # real nodes per core (ceil)
    c["SH"] = c["WIN"] * 128                          # padded nodes per core
    assert c["SH"] >= c["SH_REAL"]
    assert c["SH"] % 2 == 0
    c["SHH"] = c["SH"] // 2                           # rank-half size
    c["NTOTH"] = c["NCORES"] * c["SHH"]               # rows per half table
    assert c["NTOTH"] - 1 <= 32767, "half-table must be int16-indexable"
    c["CALLB"] = 8                                    # blocks per gather call
    return c


def _prep(cfg, x, edge_attr, W_e1, b_e1, W_e2, b_e2, W_gcn, b_gcn, W2, b2, W3, b3,
          edge_index, batch):
    """Host-side sharding/layout. Returns (in_maps, meta)."""
    N, E, G, A = cfg["N"], cfg["E"], cfg["G"], cfg["A"]
    NC, WIN, SH_REAL, SH = cfg["NCORES"], cfg["WIN"], cfg["SH_REAL"], cfg["SH"]
    SHH, CALLB = cfg["SHH"], cfg["CALLB"]

    x = np.asarray(x, np.float32)
    edge_attr = np.asarray(edge_attr, np.float32)
    edge_index = np.asarray(edge_index)
    batch = np.asarray(batch)
    src = np.asarray(edge_index[0], np.int64)
    dst = np.asarray(edge_index[1], np.int64)
    attr = edge_attr[:, 0]

    deg = np.bincount(dst, minlength=N)

    # per-core degree-sorted window/slot assignment
    node_of_rank = np.full((NC, SH), -1, np.int64)   # rank -> orig node id (-1 pad)
    rank_of_orig = np.empty(N, np.int64)             # orig -> rank within its core
    R1_cw = np.zeros((NC, WIN), np.int64)
    for c in range(NC):
        lo, hi = c * SH_REAL, min((c + 1) * SH_REAL, N)
        nreal = hi - lo
        d_loc = np.full(SH, -1, np.int64)
        d_loc[:nreal] = deg[lo:hi]
        order = np.argsort(-d_loc, kind="stable")    # rank -> padded-loc
        rank = np.empty(SH, np.int64)
        rank[order] = np.arange(SH)
        node_of_rank[c] = np.where(order < nreal, lo + order, -1)
        rank_of_orig[lo:hi] = rank[:nreal]
        R1_cw[c] = np.maximum(d_loc[order].reshape(WIN, 128), 0).max(axis=1)

    R1_w = R1_cw.max(axis=0)
    R1TOT = max(int(R1_w.sum()), 1)
    woff1 = np.zeros(WIN + 1, np.int64)
    woff1[1:] = np.cumsum(R1_w)

    core_of = np.minimum(np.arange(N) // SH_REAL, NC - 1)

    # per-edge coordinates
    ecore = np.minimum(dst // SH_REAL, NC - 1)
    erank = rank_of_orig[dst]
    ew = erank // 128
    ep = erank % 128
    score = np.minimum(src // SH_REAL, NC - 1)
    srank = rank_of_orig[src]
    ehalf = (srank >= SHH).astype(np.int64)
    erow = score * SHH + srank - ehalf * SHH         # row within half table

    # j1 = rank of edge within its dst-node's list (degree pass)
    eorder = np.argsort(dst, kind="stable")
    starts = np.zeros(N + 1, np.int64)
    starts[1:] = np.cumsum(deg)
    j1 = np.empty(E, np.int64)
    j1[eorder] = np.arange(E) - starts[dst[eorder]]

    # pass-2 segment = (window, half); per-core counts -> uniform block counts
    segid = ew * 2 + ehalf                            # 0..2*WIN-1
    cnt = np.zeros((NC, 2 * WIN), np.int64)
    for c in range(NC):
        m = ecore == c
        cnt[c] = np.bincount(segid[m], minlength=2 * WIN)
    NB_seg = -(-cnt.max(axis=0) // 128)               # blocks per segment (uniform)

    seg_boff = np.zeros(2 * WIN, np.int64)
    calls = []                                        # (half, block_start, nblocks)
    pos = 0
    for w in range(WIN):
        for h in (0, 1):
            seg_boff[w * 2 + h] = pos
            nseg = int(NB_seg[w * 2 + h])
            b = pos
            pos += nseg
            while b < pos:
                nb = min(CALLB, pos - b)
                calls.append((h, int(b), int(nb)))
                b += nb
    NBLK = max(int(pos), 1)

    # per-window block list in call order
    win_blocks = []
    for w in range(WIN):
        blks = []
        for h in (0, 1):
            b0 = int(seg_boff[w * 2 + h])
            blks.extend(range(b0, b0 + int(NB_seg[w * 2 + h])))
        win_blocks.append(blks)

    # j2 = rank of edge within its (core, segment) group
    keys = (ecore * (2 * WIN) + segid)
    eorder2 = np.argsort(keys, kind="stable")
    gcnt = np.bincount(keys, minlength=NC * 2 * WIN)
    gstarts = np.zeros(NC * 2 * WIN + 1, np.int64)
    gstarts[1:] = np.cumsum(gcnt)
    j2 = np.empty(E, np.int64)
    j2[eorder2] = np.arange(E) - gstarts[keys[eorder2]]

    we1 = np.asarray(W_e1, np.float64).reshape(3)
    be1 = float(np.asarray(b_e1, np.float64).reshape(-1)[0])
    we2 = float(np.asarray(W_e2, np.float64).reshape(-1)[0])
    be2 = float(np.asarray(b_e2, np.float64).reshape(-1)[0])
    s0_all = (we1[0] * src + we1[1] * dst + be1).astype(np.float32)

    ecv = np.array([we1[2], we2, be2, 0.0], np.float32)
    ec_bcast = np.ascontiguousarray(np.broadcast_to(ecv, (128, 4)))

    iota128 = np.ascontiguousarray(
        np.broadcast_to(np.arange(128, dtype=np.float32), (128, 128)).astype(BF16))
    bgcn_b = np.ascontiguousarray(
        np.broadcast_to(np.asarray(b_gcn, np.float32), (128, 128)))
    b3_b = np.ascontiguousarray(
        np.broadcast_to(np.asarray(b3, np.float32), (64, A)))
    ident64 = np.eye(64, dtype=BF16)
    ident128 = np.eye(128, dtype=BF16)
    wgcn_b16 = np.ascontiguousarray(np.asarray(W_gcn, np.float32)).astype(BF16)
    w2_b16 = np.ascontiguousarray(np.asarray(W2, np.float32)).astype(BF16)
    w3_b16 = np.ascontiguousarray(np.asarray(W3, np.float32)).astype(BF16)
    b2_np = np.ascontiguousarray(np.asarray(b2, np.float32).reshape(128, 1))

    in_maps = []
    for c in range(NC):
        m = ecore == c
        s_s0, s_attr = s0_all[m], attr[m]
        s_ep, s_ew, s_j1, s_j2 = ep[m], ew[m], j1[m], j2[m]
        s_seg, s_row = segid[m], erow[m]

        # degree-pass slot-layout streams [128, R1TOT]
        p1_s0 = np.zeros((128, R1TOT), np.float32)
        p1_attr = np.zeros((128, R1TOT), BF16)
        p1_mask = np.zeros((128, R1TOT), BF16)
        col1 = woff1[s_ew] + s_j1
        p1_s0[s_ep, col1] = s_s0
        p1_attr[s_ep, col1] = s_attr
        p1_mask[s_ep, col1] = 1.0

        # block-layout streams [128, NBLK]
        p2_s0 = np.zeros((128, NBLK), np.float32)
        p2_attr = np.zeros((128, NBLK), BF16)
        p2_dl = np.full((128, NBLK), -1.0, np.float32)  # pads: one-hot column dead
        blk = seg_boff[s_seg] + s_j2 // 128
        pp = s_j2 % 128
        p2_s0[pp, blk] = s_s0
        p2_attr[pp, blk] = s_attr
        p2_dl[pp, blk] = s_ep

        # gather idx stream, wrapped int16 [128, NBLK*8]; pads fetch row 0
        idx_flat = np.zeros(NBLK * 128, np.int64)
        k = blk * 128 + pp
        idx_flat[k] = s_row
        idx16 = np.zeros((128, NBLK * 8), np.int16)
        wrap = idx_flat.reshape(NBLK * 8, 16).T.astype(np.int16)
        for gg in range(8):
            idx16[gg * 16:(gg + 1) * 16, :] = wrap

        # xT in slot order [128, SH] bf16
        nr = node_of_rank[c]
        valid = nr >= 0
        xs = np.zeros((SH, x.shape[1]), np.float32)
        xs[valid] = x[nr[valid]]
        xT = np.ascontiguousarray(xs.T).astype(BF16)

        batch_slot = np.full((128, WIN), 127.0, np.float32)
        nmask = np.zeros((128, WIN), BF16)
        bvals = np.full(SH, 127, np.int64)
        bvals[valid] = batch[nr[valid]]
        batch_slot[:, :] = bvals.reshape(WIN, 128).T
        nmask[:, :] = (valid.reshape(WIN, 128).T).astype(BF16)

        in_maps.append({
            "p1_s0": p1_s0, "p1_attr": p1_attr, "p1_mask": p1_mask,
            "p2_s0": p2_s0, "p2_attr": p2_attr, "p2_dl": p2_dl, "p2_idx": idx16,
            "xT": xT, "batch_slot": batch_slot, "nmask": nmask,
            "iota": iota128, "ecb": ec_bcast, "wgcn": wgcn_b16, "bgcnb": bgcn_b,
            "w2": w2_b16, "b2": b2_np, "w3": w3_b16, "b3b": b3_b, "ident": ident64,
            "ident128": ident128,
        })

    meta = dict(R1TOT=R1TOT, R1_w=[int(v) for v in R1_w],
                woff1=[int(v) for v in woff1],
                NBLK=NBLK, calls=calls, win_blocks=win_blocks)
    return in_maps, meta


def _build(cfg, meta):
    from concourse import bass, bacc, tile
    import concourse.mybir as mybir

    f32 = mybir.dt.float32
    bf16 = mybir.dt.bfloat16
    i16 = mybir.dt.int16
    Alu = mybir.AluOpType
    Act = mybir.ActivationFunctionType

    NC, WIN, SH, SHH = cfg["NCORES"], cfg["WIN"], cfg["SH"], cfg["SHH"]
    NTOTH, G, A = cfg["NTOTH"], cfg["G"], cfg["A"]
    R1TOT, R1_w, woff1 = meta["R1TOT"], meta["R1_w"], meta["woff1"]
    NBLK, calls, win_blocks = meta["NBLK"], meta["calls"], meta["win_blocks"]

    nc = bacc.Bacc("TRN2", target_bir_lowering=False, debug=False, num_devices=NC,
                   num_swdge_queues=4)

    dram = lambda nm, shp, dt: nc.dram_tensor(nm, shp, dt, kind="ExternalInput")
    p1_s0_d = dram("p1_s0", [128, R1TOT], f32)
    p1_attr_d = dram("p1_attr", [128, R1TOT], bf16)
    p1_mask_d = dram("p1_mask", [128, R1TOT], bf16)
    p2_s0_d = dram("p2_s0", [128, NBLK], f32)
    p2_attr_d = dram("p2_attr", [128, NBLK], bf16)
    p2_dl_d = dram("p2_dl", [128, NBLK], f32)
    p2_idx_d = dram("p2_idx", [128, NBLK * 8], i16)
    xT_d = dram("xT", [128, SH], bf16)
    batch_d = dram("batch_slot", [128, WIN], f32)
    nmask_d = dram("nmask", [128, WIN], bf16)
    iota_d = dram("iota", [128, 128], bf16)
    ecb_d = dram("ecb", [128, 4], f32)
    wgcn_d = dram("wgcn", [128, 128], bf16)
    bgcnb_d = dram("bgcnb", [128, 128], f32)
    w2_d = dram("w2", [128, 128], bf16)
    b2_d = dram("b2", [128, 1], f32)
    w3_d = dram("w3", [128, A], bf16)
    b3b_d = dram("b3b", [64, A], f32)
    ident_d = dram("ident", [64, 64], bf16)
    id128_d = dram("ident128", [128, 128], bf16)
    out_d = nc.dram_tensor("out", [64, A], f32, kind="ExternalOutput")

    tabsh_d = nc.dram_tensor("tabsh", [SH, 128], bf16)
    tabA_d = nc.dram_tensor("tabA", [NTOTH, 128], bf16, addr_space="Shared")
    tabB_d = nc.dram_tensor("tabB", [NTOTH, 128], bf16, addr_space="Shared")
    pool_in_d = nc.dram_tensor("pool_in", [64, 129], f32)
    pool_out_d = nc.dram_tensor("pool_out", [64, 129], f32, addr_space="Shared")

    groups = [list(range(NC))]
    sq_fn = getattr(Act, "Square")

    with tile.TileContext(nc) as tc:
        with (
            tc.tile_pool(name="const", bufs=1) as cpool,
            tc.tile_pool(name="work", bufs=1) as wpool,
            tc.tile_pool(name="mtile", bufs=28) as mpool,
            tc.tile_pool(name="small", bufs=12) as spool,
            tc.tile_pool(name="hone", bufs=3) as hpool,
            tc.tile_pool(name="psA", bufs=3, space="PSUM") as psA,
            tc.tile_pool(name="psB", bufs=1, space="PSUM") as psB,
            tc.tile_pool(name="psC", bufs=1, space="PSUM") as psC,
        ):
            # ---- constants ----
            iota_t = cpool.tile([128, 128], bf16)
            ec_t = cpool.tile([128, 4], f32)
            bgcn_t = cpool.tile([128, 128], f32)
            nmask_t = cpool.tile([128, WIN], bf16)
            batch_t = cpool.tile([128, WIN], f32)
            wgcn_t = cpool.tile([128, 128], bf16)
            nc.sync.dma_start(out=iota_t[:], in_=iota_d[:])
            nc.sync.dma_start(out=ec_t[:], in_=ecb_d[:])
            nc.sync.dma_start(out=bgcn_t[:], in_=bgcnb_d[:])
            nc.sync.dma_start(out=nmask_t[:], in_=nmask_d[:])
            nc.sync.dma_start(out=batch_t[:], in_=batch_d[:])
            nc.sync.dma_start(out=wgcn_t[:], in_=wgcn_d[:])
            id128_t = cpool.tile([128, 128], bf16)
            nc.sync.dma_start(out=id128_t[:], in_=id128_d[:])
            bgcnb_t = cpool.tile([128, 128], bf16)
            nc.vector.tensor_copy(out=bgcnb_t[:], in_=bgcn_t[:])

            # ---- local xw table (unscaled yet), kept in SBUF ----
            locall = wpool.tile([128, SH], bf16)
            loc = [locall[:, w * 128:(w + 1) * 128] for w in range(WIN)]
            with tc.tile_pool(name="xt", bufs=1) as xpool:
                xtall = xpool.tile([128, SH], bf16)
                nc.sync.dma_start(out=xtall[:], in_=xT_d[:])
                for w in range(WIN):
                    ps = psA.tile([128, 128], f32, tag="mm")
                    nc.tensor.matmul(ps[:], xtall[:, w * 128:(w + 1) * 128],
                                     wgcn_t[:], start=True, stop=True)
                    nc.scalar.activation(out=loc[w], in_=ps[:], func=Act.Copy)

            # ---- degree pass: edge MLP + deg + dis ----
            with tc.tile_pool(name="p1", bufs=1) as p1:
                p1s0 = p1.tile([128, R1TOT], f32)
                p1at = p1.tile([128, R1TOT], bf16)
                p1mk = p1.tile([128, R1TOT], bf16)
                nc.sync.dma_start(out=p1s0[:], in_=p1_s0_d[:])
                nc.sync.dma_start(out=p1at[:], in_=p1_attr_d[:])
                nc.sync.dma_start(out=p1mk[:], in_=p1_mask_d[:])
                h_t = p1.tile([128, R1TOT], f32)
                nc.vector.scalar_tensor_tensor(out=h_t[:], in0=p1at[:],
                                               scalar=ec_t[:, 0:1], in1=p1s0[:],
                                               op0=Alu.mult, op1=Alu.add)
                nc.scalar.activation(out=h_t[:], in_=h_t[:], func=Act.Relu)
                wp = p1.tile([128, R1TOT], f32)
                nc.scalar.activation(out=wp[:], in_=h_t[:], func=Act.Sigmoid,
                                     bias=ec_t[:, 2:3], scale=ec_t[:, 1:2])
                nc.vector.tensor_tensor(out=wp[:], in0=wp[:], in1=p1mk[:],
                                        op=Alu.mult)
                deg_t = wpool.tile([128, WIN], f32)
                for w in range(WIN):
                    if R1_w[w] > 0:
                        nc.vector.tensor_reduce(
                            out=deg_t[:, w:w + 1],
                            in_=wp[:, woff1[w]:woff1[w] + R1_w[w]],
                            axis=mybir.AxisListType.X, op=Alu.add)
                    else:
                        nc.vector.memset(deg_t[:, w:w + 1], 0.0)
                nc.vector.tensor_scalar(out=deg_t[:], in0=deg_t[:], scalar1=1.0,
                                        scalar2=None, op0=Alu.add)
                sq_t = wpool.tile([128, WIN], f32)
                nc.scalar.activation(out=sq_t[:], in_=deg_t[:], func=Act.Sqrt)
                dis_t = wpool.tile([128, WIN], f32)
                nc.vector.reciprocal(out=dis_t[:], in_=sq_t[:])

            # ---- scale table by dis, write out, AllGather halves ----
            def wr_tab(w0, w1):
                nw = w1 - w0
                nc.sync.dma_start(
                    out=tabsh_d[w0 * 128:w1 * 128, :].rearrange(
                        "(w p) f -> p w f", p=128),
                    in_=locall[:, w0 * 128:w1 * 128].rearrange(
                        "p (w f) -> p w f", f=128))
            WHALF = (SHH + 127) // 128                # windows covering half A
            for w in range(WIN):
                nc.vector.tensor_scalar(out=loc[w], in0=loc[w],
                                        scalar1=dis_t[:, w:w + 1], scalar2=None,
                                        op0=Alu.mult)
                if w == WHALF - 1:
                    for a in range(0, WHALF, 7):
                        wr_tab(a, min(a + 7, WHALF))
                    nc.gpsimd.collective_compute(
                        "AllGather", Alu.bypass, replica_groups=groups,
                        ins=[tabsh_d[0:SHH, :]], outs=[tabA_d[:]])
            for a in range(WHALF, WIN, 7):
                wr_tab(a, min(a + 7, WIN))
            nc.gpsimd.collective_compute(
                "AllGather", Alu.bypass, replica_groups=groups,
                ins=[tabsh_d[SHH:SH, :]], outs=[tabB_d[:]])

            # ---- block-layout MLP ----
            p2dl = wpool.tile([128, NBLK], f32)
            nc.sync.dma_start(out=p2dl[:], in_=p2_dl_d[:])
            idx_t = wpool.tile([128, NBLK * 8], i16)
            nc.sync.dma_start(out=idx_t[:], in_=p2_idx_d[:])
            w2s = wpool.tile([128, NBLK], f32)
            negw = wpool.tile([128, NBLK], f32)
            ndl = wpool.tile([128, NBLK], f32)
            with tc.tile_pool(name="p2", bufs=1) as p2:
                p2s0 = p2.tile([128, NBLK], f32)
                p2at = p2.tile([128, NBLK], bf16)
                nc.sync.dma_start(out=p2s0[:], in_=p2_s0_d[:])
                nc.sync.dma_start(out=p2at[:], in_=p2_attr_d[:])
                h2t = p2.tile([128, NBLK], f32)
                nc.vector.scalar_tensor_tensor(out=h2t[:], in0=p2at[:],
                                               scalar=ec_t[:, 0:1], in1=p2s0[:],
                                               op0=Alu.mult, op1=Alu.add)
                nc.scalar.activation(out=h2t[:], in_=h2t[:], func=Act.Relu)
                nc.scalar.activation(out=w2s[:], in_=h2t[:], func=Act.Sigmoid,
                                     bias=ec_t[:, 2:3], scale=ec_t[:, 1:2])
            nc.vector.tensor_scalar(out=negw[:], in0=w2s[:], scalar1=-1.0,
                                    scalar2=None, op0=Alu.mult)
            nc.vector.tensor_scalar(out=ndl[:], in0=p2dl[:], scalar1=-1.0,
                                    scalar2=None, op0=Alu.mult)

            # ---- spine: gathers on 4 queues ----
            blk_tile = {}
            for ci, (h, b0, nb) in enumerate(calls):
                mt = mpool.tile([128, nb, 128], bf16, tag="M")
                tab = tabA_d if h == 0 else tabB_d
                nc.gpsimd.dma_gather(
                    out_ap=mt[:],
                    in_ap=tab[:],
                    idxs_ap=idx_t[:, b0 * 8:(b0 + nb) * 8],
                    num_idxs=nb * 128,
                    num_idxs_reg=nb * 128,
                    elem_size=128,
                    single_packet=False,
                    queue_num=ci % 4,
                )
                for i in range(nb):
                    blk_tile[b0 + i] = (mt, i)

            # ---- scatter matmuls + (delayed) finalize per window ----
            pool_ps = psB.tile([64, 129], f32, tag="poolps")

            def emit_fin(w, psw):
                h1 = hpool.tile([128, 129], bf16, tag="h1")
                nc.scalar.activation(out=h1[:, 0:128], in_=psw[:], func=Act.Relu,
                                     scale=dis_t[:, w:w + 1])
                nc.vector.tensor_copy(out=h1[:, 128:129], in_=nmask_t[:, w:w + 1])
                pw = spool.tile([128, 64], bf16, tag="pw")
                nc.vector.tensor_scalar(
                    out=pw[:], in0=iota_t[:, 0:64],
                    scalar1=batch_t[:, w:w + 1], scalar2=None, op0=Alu.is_equal)
                nc.tensor.matmul(pool_ps[:], pw[:], h1[:],
                                 start=(w == 0), stop=(w == WIN - 1))

            gb = 0
            for w in range(WIN):
                blks = win_blocks[w]
                psw = psA.tile([128, 128], f32, tag="mm")
                for bi, b in enumerate(blks):
                    mt, i = blk_tile[b]
                    if gb % 4 == 3:
                        ab = spool.tile([128, 128], f32, tag="ab")
                        nc.scalar.activation(out=ab[:], in_=iota_t[:], func=sq_fn,
                                             bias=ndl[:, b:b + 1], scale=1.0)
                        s_t = spool.tile([128, 128], bf16, tag="sA")
                        nc.scalar.activation(out=s_t[:], in_=ab[:], func=Act.Relu,
                                             bias=w2s[:, b:b + 1],
                                             scale=negw[:, b:b + 1])
                    else:
                        s_t = spool.tile([128, 128], bf16, tag="sV")
                        nc.vector.tensor_scalar(
                            out=s_t[:], in0=iota_t[:],
                            scalar1=p2dl[:, b:b + 1], scalar2=w2s[:, b:b + 1],
                            op0=Alu.is_equal, op1=Alu.mult)
                    gb += 1
                    nc.tensor.matmul(psw[:], s_t[:], mt[:, i, :],
                                     start=(bi == 0), stop=False)
                # self-loop rows (identity @ local table) and bias row
                # (diag(sqrt(deg)) @ bgcn broadcast): psw += loc + sqrt(deg)*b
                nc.tensor.matmul(psw[:], id128_t[:], loc[w], start=False, stop=False)
                dgs = spool.tile([128, 128], bf16, tag="dgs")
                nc.vector.tensor_scalar(out=dgs[:], in0=id128_t[:],
                                        scalar1=sq_t[:, w:w + 1], scalar2=None,
                                        op0=Alu.mult)
                nc.tensor.matmul(psw[:], dgs[:], bgcnb_t[:], start=False, stop=True)
                emit_fin(w, psw)

            # ---- AllReduce pooled ----
            pool_sb = wpool.tile([64, 129], f32)
            nc.vector.tensor_copy(out=pool_sb[:], in_=pool_ps[:])
            nc.sync.dma_start(out=pool_in_d[:], in_=pool_sb[:])
            nc.gpsimd.collective_compute(
                "AllReduce", Alu.add, replica_groups=groups,
                ins=[pool_in_d[:]], outs=[pool_out_d[:]])
            pool2 = wpool.tile([64, 129], f32)
            nc.sync.dma_start(out=pool2[:], in_=pool_out_d[:])

            # ---- head ----
            cntm = wpool.tile([64, 1], f32)
            nc.vector.tensor_scalar(out=cntm[:], in0=pool2[:, 128:129], scalar1=1.0,
                                    scalar2=None, op0=Alu.max)
            rec = wpool.tile([64, 1], f32)
            nc.vector.reciprocal(out=rec[:], in_=cntm[:])
            pooled_b = wpool.tile([64, 128], bf16)
            nc.vector.tensor_scalar(out=pooled_b[:], in0=pool2[:, 0:128],
                                    scalar1=rec[:], scalar2=None, op0=Alu.mult)

            ident_t = cpool.tile([64, 64], bf16)
            nc.sync.dma_start(out=ident_t[:], in_=ident_d[:])
            psT = psC.tile([128, 64], bf16, tag="pT")
            nc.tensor.transpose(psT[:], pooled_b[:], ident_t[:])
            pooledT = wpool.tile([128, 64], bf16)
            nc.vector.tensor_copy(out=pooledT[:], in_=psT[:])

            w2b = cpool.tile([128, 128], bf16)
            nc.sync.dma_start(out=w2b[:], in_=w2_d[:])
            b2_t = cpool.tile([128, 1], f32)
            nc.sync.dma_start(out=b2_t[:], in_=b2_d[:])
            h2ps = psC.tile([128, 64], f32, tag="h2")
            nc.tensor.matmul(h2ps[:], w2b[:], pooledT[:], start=True, stop=True)
            h2sb = wpool.tile([128, 64], bf16)
            nc.scalar.activation(out=h2sb[:], in_=h2ps[:], func=Act.Relu,
                                 bias=b2_t[:], scale=1.0)

            w3b = cpool.tile([128, A], bf16)
            nc.sync.dma_start(out=w3b[:], in_=w3_d[:])
            b3_t = cpool.tile([64, A], f32)
            nc.sync.dma_start(out=b3_t[:], in_=b3b_d[:])
            yps = psC.tile([64, A], f32, tag="y")
            nc.tensor.matmul(yps[:], h2sb[:], w3b[:], start=True, stop=True)
            ysb = wpool.tile([64, A], f32)
            nc.vector.tensor_tensor(out=ysb[:], in0=yps[:], in1=b3_t[:], op=Alu.add)
            nc.sync.dma_start(out=out_d[:], in_=ysb[:])

    nc.compile()
    return nc


_CACHE = {}


def _get_program(cfg, meta):
    key = (tuple(sorted(cfg.items())), meta["R1TOT"], tuple(meta["R1_w"]),
           meta["NBLK"], tuple(meta["calls"]),
           tuple(tuple(b) for b in meta["win_blocks"]))
    if key not in _CACHE:
        _CACHE[key] = _build(cfg, meta)
    return _CACHE[key]


def kernel(**inputs):
    from concourse import bass_utils
    cfg = _derived(_default_cfg())
    inputs = {k: np.asarray(v) for k, v in inputs.items()}
    in_maps, meta = _prep(cfg, **inputs)
    nc = _get_program(cfg, meta)
    res = bass_utils.run_bass_kernel_spmd(nc, in_maps, list(range(cfg["NCORES"])))
    return np.asarray(res.results[0]["out"], np.float32)[: cfg["G"]]
